# revision 42
# baseline (speedup 1.0000x reference)
"""Causal MHA with RoPE on 8 Trainium2 NeuronCores.

Sharding: core c -> batch b=c//2, head-group g=c%2 (8 heads of 16).
Each core: Q/K/V projections for its 512 head-dims over the full sequence,
causal attention for its 8 heads, partial output projection (its 512 rows
of wo). Host sums the two partial outputs per batch. No collectives.

All operands bf16 (fp32 PSUM accumulation), prepared host-side:
 - x^T materialized by XBAR DMA-transpose straight into SBUF (no PE work).
 - Weights/tables loaded once, DMA order latency-tuned (the scheduler
   chains coarsened waits between nearby DMAs, so transfer sizes are kept
   small and ordered by first use).
 - RoPE: dst = C*acc + PM@(S*acc), PM a 32-row block-swap permutation
   matrix as a PE matmul (no SBUF swap DMAs); sign of S folded host-side;
   each chunk's rope tail is emitted after the next chunk's matmuls so the
   PE never waits on it.
 - Causal mask: exp first (ScalarE, scale=1/8 folded in), then one bf16
   DVE multiply of the diagonal 128-tile by a 0/1 lower-triangular mask.
 - AV computed transposed: exp-block stationary, [V | 1] moving ->
   ctx^T [q, dim] at 65 cols per (tile, head) instead of streaming exp
   twice; the ones column yields softmax denominators for free. Each
   (pair, query-chunk) accumulation group is contiguous and owns its PSUM
   tile: interleaved groups within one tile corrupt on hardware.
 - ctx^T scaled by 1/denom (per-partition scalar), transposed back to
   [dim, tok] by XBAR DMA for the output projection; bf16 output summed
   across head-group cores on the host.
Issue order interleaves projections of pass t+1 and the output projection
into the attention stream of pass t (weighted toward the exp-bound prefix
iterations) so ScalarE exp time hides behind PE work.
Timeline-sim: 245973 ns/core (baseline 413016); rel err vs fp32 ref 3.7e-3.
"""
import math
import os

import numpy as np

import concourse.bass as bass
import concourse.mybir as mybir
import concourse.tile as tile
from concourse import bacc
from concourse.bass_utils import run_bass_kernel_spmd

F32 = mybir.dt.float32
BF16 = mybir.dt.bfloat16

B, S, D, H = 4, 2048, 1024, 16
HD = D // H          # 64
THETA = 10000.0
DH = D // 2          # 512 per-core head dims (8 heads)
NP = 4               # head pairs per core
NTH = 4              # token passes
THT = S // NTH       # 512 tokens per pass
QB = THT             # query block
NKT = S // 128       # 16 key tiles of 128
SCALE = 1.0 / math.sqrt(HD)

_cached = None


def _build():
    nc = bacc.Bacc(None, target_bir_lowering=False)

    x = nc.dram_tensor("x", [S, D], BF16, kind="ExternalInput")
    wq = nc.dram_tensor("wq", [128, 8, DH], BF16, kind="ExternalInput")
    wk = nc.dram_tensor("wk", [128, 8, DH], BF16, kind="ExternalInput")
    wv = nc.dram_tensor("wv", [128, 8, DH], BF16, kind="ExternalInput")
    wo = nc.dram_tensor("wo", [128, 4, D], BF16, kind="ExternalInput")
    cosb = nc.dram_tensor("cosb", [128, S], BF16, kind="ExternalInput")
    sinb = nc.dram_tensor("sinb", [128, S], BF16, kind="ExternalInput")
    # [PM | ident | tri01] host-built constants
    consts = nc.dram_tensor("consts", [128, 3, 128], BF16, kind="ExternalInput")
    outp = nc.dram_tensor("outp", [S, D], BF16, kind="ExternalOutput")
    dbg = {}
    if os.environ.get("KDBG"):
        for nm, shp in (("dxt0", [128, S]), ("dkt0", [128, S]), ("dqt0", [128, S]),
                        ("dv0", [128, 8, HD + 1]), ("dctx0", [128, S]),
                        ("deab", [128, 2 * QB]), ("dpse", [128, 4, HD + 1]),
                        ("dctxT", [128, 4, 128])):
            dbg[nm] = nc.dram_tensor(nm, shp, BF16, kind="ExternalOutput")

    with tile.TileContext(nc) as tc:
        with (
            tc.tile_pool(name="const", bufs=1) as cpool,
            tc.tile_pool(name="xt", bufs=1) as xpool,
            tc.tile_pool(name="kq", bufs=1) as kqpool,
            tc.tile_pool(name="vaug", bufs=1) as vpool,
            tc.tile_pool(name="wts", bufs=1) as wpool,
            tc.tile_pool(name="stream", bufs=2) as spool,
        ):
            # weights, loaded once on the Pool queue (parallel to sync queue)
            wq_s = wpool.tile([128, 8, DH], BF16, name="wq_s")
            wk_s = wpool.tile([128, 8, DH], BF16, name="wk_s")
            wv_s = wpool.tile([128, 8, DH], BF16, name="wv_s")
            wo_s = wpool.tile([128, 4, D], BF16, name="wo_s")
            # All loads go through the in-order SP queue: the scheduler's
            # coarsened cross-queue DMA waits serialize arbitrary pairs, so
            # explicit FIFO placement beats a second queue. Weights split in
            # 0.5MB chunks to keep any one hold on the DMA engines short.
            # wk first: K-projection chunks are emitted before Q's.
            def wload(dst, src):
                n = dst.shape[1]
                for c in range(0, n, n // 2):
                    nc.sync.dma_start(out=dst[:, c : c + n // 2, :],
                                      in_=src[:, c : c + n // 2, :])

            ctile = cpool.tile([128, 3, 128], BF16, name="ctile")
            cos_t = cpool.tile([128, S], BF16, name="cos_t")
            sin_t = cpool.tile([128, S], BF16, name="sin_t")
            pmat = ctile[:, 0, :]
            tri01 = ctile[:, 2, :]

            # x^T tiles: xt[dc] = [128 dims, S tokens], via XBAR DMA transpose.
            # The DMA order is latency-tuned: the scheduler adds coarsened
            # waits chaining each DMA to one a few slots earlier (even across
            # queues), so big transfers are interleaved between the x^T
            # chunks in the order compute first needs them.
            xt = [xpool.tile([128, S], BF16, name=f"xt{dc}") for dc in range(8)]

            def xtload(th, dc):
                t0 = th * THT
                nc.sync.dma_start_transpose(
                    out=xt[dc][:, t0 : t0 + THT],
                    in_=x[t0 : t0 + THT, dc * 128 : (dc + 1) * 128],
                )

            def half(dst, src, h):
                nc.sync.dma_start(out=dst[:, h * (S // 2) : (h + 1) * (S // 2)],
                                  in_=src[:, h * (S // 2) : (h + 1) * (S // 2)])

            nc.gpsimd.dma_start(out=ctile, in_=consts[:, :, :])
            half(cos_t, cosb, 0)
            half(sin_t, sinb, 0)
            wload(wk_s, wk)           # wk0, wk1
            for dc in range(8):
                xtload(0, dc)
            half(cos_t, cosb, 1)
            half(sin_t, sinb, 1)
            late_w = {0: (wq_s, wq), 1: (wv_s, wv), 2: (wo_s, wo)}
            for th in range(NTH):
                if th > 0:
                    for dc in range(8):
                        xtload(th, dc)
                if th in late_w:
                    dst, src = late_w[th]
                    wload(dst, src)

            # K^T / Q^T pair tiles: [128 dims (head 2p | head 2p+1), S tokens]
            kt_tiles = [kqpool.tile([128, S], BF16, name=f"ktp{p}") for p in range(NP)]
            qt_tiles = [kqpool.tile([128, S], BF16, name=f"qtp{p}") for p in range(NP)]
            ctx_tiles = [kqpool.tile([128, S], BF16, name=f"ctxp{p}") for p in range(NP)]
            # V tiles with ones column: [128 tokens, 8 heads, 64+1]
            v_tiles = [vpool.tile([128, 8, HD + 1], BF16, name=f"vt{t}") for t in range(NKT)]
            for t in range(NKT):
                # ones column via exp(0*x) = 1
                nc.scalar.activation(
                    v_tiles[t][:, :, HD], ctile[:, 0, 0:8],
                    mybir.ActivationFunctionType.Exp, scale=0.0,
                )

            with (
                tc.tile_pool(name="pst", bufs=2, space="PSUM") as pst,
                tc.tile_pool(name="pssc", bufs=2, space="PSUM") as pssc,
                tc.tile_pool(name="psav", bufs=2, space="PSUM") as psav,
            ):
                # ---------- work-item generators ----------
                def proj_chunks(th):
                    t0 = th * THT
                    ts = slice(t0, t0 + THT)

                    def qk_mms(wsb, dst, p):
                        # returns the rope-tail closure; caller emits it after
                        # the NEXT chunk's matmuls so the PE never waits on it
                        acc = pst.tile([128, THT], F32, name="acc", tag="tp")
                        for dc in range(8):
                            nc.tensor.matmul(
                                acc, wsb[:, dc, p * 128 : (p + 1) * 128],
                                xt[dc][:, ts],
                                start=(dc == 0), stop=(dc == 7),
                            )
                        acc_sb = spool.tile([128, THT], BF16, name="acc_sb",
                                            tag="accsb", bufs=4)
                        nc.scalar.copy(acc_sb, acc)  # frees the PSUM slot fast
                        sacc = spool.tile([128, THT], BF16, name="sacc",
                                          tag="sacc", bufs=3)
                        nc.vector.tensor_mul(sacc, acc_sb, sin_t[:, ts])

                        def rope_tail():
                            # dst = C*acc + PM@(S*acc), S sign-folded host-side
                            rps = pst.tile([128, THT], F32, name="rps", tag="tp")
                            nc.tensor.matmul(rps, pmat, sacc)
                            t1 = spool.tile([128, THT], BF16, name="t1",
                                            tag="t1", bufs=3)
                            nc.vector.tensor_mul(t1, acc_sb, cos_t[:, ts])
                            nc.vector.tensor_add(dst[p][:, ts], t1, rps)
                        return rope_tail

                    chunks = []
                    tail_box = [None]
                    for wsb, dst in ((wk_s, kt_tiles), (wq_s, qt_tiles)):
                        for p in range(NP):
                            def qk_chunk(wsb=wsb, dst=dst, p=p, tail_box=tail_box):
                                prev = tail_box[0]
                                tail_box[0] = qk_mms(wsb, dst, p)
                                if prev is not None:
                                    prev()
                            chunks.append(qk_chunk)
                    for tl in range(THT // 128):
                        def v_chunk(tl=tl, t0=t0, th=th, tail_box=tail_box):
                            acc = pst.tile([128, DH], F32, name="vacc", tag="tp")
                            for dc in range(8):
                                nc.tensor.matmul(
                                    acc, xt[dc][:, t0 + tl * 128 : t0 + (tl + 1) * 128],
                                    wv_s[:, dc, :],
                                    start=(dc == 0), stop=(dc == 7),
                                )
                            prev = tail_box[0]
                            tail_box[0] = None
                            if prev is not None:
                                prev()
                            vt = v_tiles[th * (THT // 128) + tl]
                            nc.vector.tensor_copy(
                                vt[:, :, 0:HD],
                                acc.rearrange("a (h d) -> a h d", h=8),
                            )
                        chunks.append(v_chunk)
                    return chunks

                def attn_iters(qb):
                    nk = 4 * qb + 4
                    q0 = qb * QB
                    iters = []

                    def av_group(p, qb, qci, eabs, ctxT_box):
                        # one contiguous accumulation group per (p, qci, head):
                        # the tile framework / PSUM HW mishandles interleaved
                        # groups within one tile, so never interleave them.
                        j = 4 * qb + qci
                        pseq = psav.tile([128, HD + 1], F32, name="pseq", tag="av")
                        psoq = psav.tile([128, HD + 1], F32, name="psoq", tag="av")
                        for kt2 in range(j + 1):
                            dj2 = max(0, kt2 - 4 * qb)
                            e0 = (qci - dj2) * 128
                            eab2 = eabs[kt2]
                            nc.tensor.matmul(
                                pseq, eab2[:, e0 : e0 + 128],
                                v_tiles[kt2][:, 2 * p, :],
                                start=(kt2 == 0), stop=(kt2 == j))
                            nc.tensor.matmul(
                                psoq, eab2[:, QB + e0 : QB + e0 + 128],
                                v_tiles[kt2][:, 2 * p + 1, :],
                                start=(kt2 == 0), stop=(kt2 == j))
                        rec = spool.tile([128, 2], F32, name="rec", tag="rec", bufs=4)
                        nc.vector.reciprocal(rec[:, 0:1], pseq[:, HD : HD + 1])
                        nc.vector.reciprocal(rec[:, 1:2], psoq[:, HD : HD + 1])
                        ctxT = spool.tile([128, 128], BF16, name="ctxT",
                                          tag="ctxT", bufs=8)
                        nc.vector.tensor_scalar_mul(ctxT[:, 0:HD], pseq[:, 0:HD],
                                                    rec[:, 0:1])
                        nc.vector.tensor_scalar_mul(ctxT[:, HD:128], psoq[:, 0:HD],
                                                    rec[:, 1:2])
                        tq = (4 * qb + qci) * 128
                        nc.sync.dma_start_transpose(
                            out=ctx_tiles[p][:, tq : tq + 128], in_=ctxT)

                    for p in range(NP):
                        iters.append([])
                        eabs = {}
                        ctxT_box = [None]
                        for j in range(nk):
                            def kt_iter(p=p, j=j, qb=qb, q0=q0, eabs=eabs,
                                        ctxT_box=ctxT_box):
                                kt = j
                                dj = kt - 4 * qb
                                qoff = 128 * dj if dj > 0 else 0
                                n = QB - qoff
                                ktp, qtp = kt_tiles[p], qt_tiles[p]
                                ksl = slice(kt * 128, (kt + 1) * 128)
                                qsl = slice(q0 + qoff, q0 + QB)
                                psab = pssc.tile([128, 2 * QB], F32, name="psab", tag="sc")
                                nc.tensor.matmul(psab[:, 0:n], ktp[0:64, ksl], qtp[0:64, qsl])
                                nc.tensor.matmul(psab[:, QB : QB + n],
                                                 ktp[64:128, ksl], qtp[64:128, qsl])
                                eab = spool.tile([128, 2 * QB], BF16, name="eab",
                                                 tag="eab", bufs=24)
                                eabs[kt] = eab
                                eview_o = bass.AP(
                                    tensor=eab.tensor, offset=eab.offset,
                                    ap=[eab.ap[0], [QB, 2], [1, n]])
                                eview_i = bass.AP(
                                    tensor=psab.tensor, offset=psab.offset,
                                    ap=[psab.ap[0], [QB, 2], [1, n]])
                                nc.scalar.activation(
                                    eview_o, eview_i,
                                    mybir.ActivationFunctionType.Exp, scale=SCALE)
                                if dj >= 0:
                                    # zero exp above the diagonal of the 128-chunk
                                    mview = bass.AP(
                                        tensor=eab.tensor, offset=eab.offset,
                                        ap=[eab.ap[0], [QB, 2], [1, 128]])
                                    tview = bass.AP(
                                        tensor=tri01.tensor, offset=tri01.offset,
                                        ap=[tri01.ap[0], [0, 2], [1, 128]])
                                    nc.vector.tensor_mul(mview, mview, tview)
                                if dbg and qb == 0 and p == 0 and kt == 0:
                                    nc.sync.dma_start(out=dbg["deab"][:, :], in_=eab[:, :])
                                # AV group one iteration behind: its last eab
                                # is already exp'd, so the PE never waits
                                if j - 1 >= 4 * qb:
                                    av_group(p, qb, j - 1 - 4 * qb, eabs, ctxT_box)
                            nmm = 2 * j if j - 1 >= 4 * qb else 0
                            npe = (2 * (QB - (128 * (j - 4 * qb) if j > 4 * qb else 0))
                                   + nmm * (HD + 1)) * 4.167e-4
                            nact = 2 * (QB - (128 * (j - 4 * qb) if j > 4 * qb else 0)) \
                                * 8.33e-4 + 0.37
                            iters[-1].append((kt_iter, npe, nact))

                        def p_flush(p=p, qb=qb, eabs=eabs, ctxT_box=ctxT_box):
                            av_group(p, qb, 3, eabs, ctxT_box)
                        iters[-1].append((p_flush, 2 * nk * (HD + 1) * 4.167e-4, 0.0))
                    return iters

                def oproj_chunk(t, nn):
                    def o_chunk(t=t, nn=nn):
                        oacc = pst.tile([128, 512], F32, name="oacc", tag="tp")
                        for pc in range(4):
                            nc.tensor.matmul(
                                oacc, ctx_tiles[pc][:, t * 128 : (t + 1) * 128],
                                wo_s[:, pc, nn * 512 : (nn + 1) * 512],
                                start=(pc == 0), stop=(pc == 3))
                        osb = spool.tile([128, 512], BF16, name="osb",
                                         tag="osb", bufs=6)
                        nc.vector.tensor_copy(osb, oacc)
                        nc.sync.dma_start(
                            out=outp[t * 128 : (t + 1) * 128,
                                     nn * 512 : (nn + 1) * 512], in_=osb)
                    return o_chunk

                # ---------- schedule ----------
                # qb=2/qb=3 attention units interleave so the exp-heavy tail
                # shares a region with enough PE filler; proj(th) chunks are
                # deadline-scheduled before attn(th) starts, out-proj tiles
                # gated on their query block's last pair finishing.
                def interleave(primary, filler):
                    # spread filler proportionally to each item's PE slack
                    wts = [1.0 if a - p_ > 0.5 else 0.25 for _, p_, a in primary]
                    total = sum(wts)
                    nf = len(filler)
                    fi, acc = 0, 0.0
                    for (fn, _, _), w in zip(primary, wts):
                        fn()
                        acc += w
                        while fi < min(int(acc / total * nf), nf):
                            filler[fi]()
                            fi += 1
                    while fi < nf:
                        filler[fi]()
                        fi += 1

                for c in proj_chunks(0):
                    c()
                aitems = {qb: attn_iters(qb) for qb in range(4)}

                def flat(qb):
                    return [it for pl in aitems[qb] for it in pl]

                for th in (1, 2, 3):
                    interleave(flat(th - 1), proj_chunks(th))
                interleave(flat(3),
                           [oproj_chunk(t, nn) for t in range(12) for nn in (0, 1)])
                for t in range(12, 16):
                    for nn in (0, 1):
                        oproj_chunk(t, nn)()

                if dbg:
                    nc.sync.dma_start(out=dbg["dxt0"][:, :], in_=xt[0][:, :])
                    nc.sync.dma_start(out=dbg["dkt0"][:, :], in_=kt_tiles[0][:, :])
                    nc.sync.dma_start(out=dbg["dqt0"][:, :], in_=qt_tiles[0][:, :])
                    nc.sync.dma_start(out=dbg["dv0"][:, :, :], in_=v_tiles[0][:, :, :])
                    nc.sync.dma_start(out=dbg["dctx0"][:, :], in_=ctx_tiles[0][:, :])

    nc.compile()
    return nc


def _host_tables(token_positions):
    pos = np.asarray(token_positions, dtype=np.float64)
    inv_freq = np.exp(np.arange(0, HD, 2, dtype=np.float64) * (-math.log(THETA) / HD))
    ang = pos[:, None] * inv_freq[None, :]  # [S, 32]
    cos = np.cos(ang).T  # [32, S]
    sin = np.sin(ang).T
    # pair-tile row layout: [head_even: 32 evens | 32 odds][head_odd: same]
    # sign folded so rope = C*acc + PM@(Sx*acc)
    C = np.empty((128, S), np.float64)
    Sx = np.empty((128, S), np.float64)
    for half in range(2):
        r0 = 64 * half
        C[r0 : r0 + 32] = cos
        C[r0 + 32 : r0 + 64] = cos
        Sx[r0 : r0 + 32] = sin
        Sx[r0 + 32 : r0 + 64] = -sin
    return C, Sx


def _host_consts():
    pm = np.zeros((128, 128), np.float64)
    for i in range(128):
        pm[i, i ^ 32] = 1.0
    ident = np.eye(128)
    tri = (np.arange(128)[None, :] >= np.arange(128)[:, None]).astype(np.float64)
    return np.stack([pm, ident, tri], axis=1)  # [128, 3, 128]


def kernel(in_features, token_positions, wq, wk, wv, wo):
    global _cached
    if _cached is None:
        _cached = _build()
    nc = _cached

    from ml_dtypes import bfloat16

    x = np.asarray(in_features, dtype=np.float32)
    # permute wq/wk columns within each head: [evens | odds]
    perm = np.concatenate(
        [64 * h + np.concatenate([np.arange(0, 64, 2), np.arange(1, 64, 2)]) for h in range(H)])
    wqp = np.asarray(wq, np.float32)[:, perm]
    wkp = np.asarray(wk, np.float32)[:, perm]
    wv = np.asarray(wv, np.float32)
    wo = np.asarray(wo, np.float32)
    C, Sx = _host_tables(token_positions)
    consts = _host_consts().astype(bfloat16)
    Cb = C.astype(bfloat16)
    Sb = Sx.astype(bfloat16)

    def wlayout(w):  # [1024, 512] -> [128, 8, 512] chunk-major
        return np.ascontiguousarray(
            w.reshape(8, 128, DH).transpose(1, 0, 2).astype(bfloat16))

    in_maps = []
    for c in range(8):
        b, g = c // 2, c % 2
        sl = slice(g * DH, (g + 1) * DH)
        wo_core = wo[sl, :]  # [512, 1024]
        in_maps.append({
            "x": np.ascontiguousarray(x[b].astype(bfloat16)),
            "wq": wlayout(wqp[:, sl]),
            "wk": wlayout(wkp[:, sl]),
            "wv": wlayout(wv[:, sl]),
            "wo": np.ascontiguousarray(
                wo_core.reshape(4, 128, D).transpose(1, 0, 2).astype(bfloat16)),
            "cosb": Cb,
            "sinb": Sb,
            "consts": consts,
        })
    results = _run(nc, in_maps)
    out = np.empty((B, S, D), np.float32)
    for b in range(B):
        out[b] = (results[2 * b]["outp"].astype(np.float32)
                  + results[2 * b + 1]["outp"].astype(np.float32))
    return out


_jit_cache = None


def _run(nc, in_maps):
    """Run the SPMD program on 8 cores, caching the jitted executable across
    calls (run_bass_kernel_spmd retraces every call). Falls back to the
    library path on any failure."""
    global _jit_cache
    try:
        import jax
        from jax.sharding import Mesh, PartitionSpec
        from jax.experimental.shard_map import shard_map
        from concourse import bass2jax
        import concourse.mybir as mybir

        if _jit_cache is None:
            bass2jax.install_neuronx_cc_hook()
            pid_name = nc.partition_id_tensor.name if nc.partition_id_tensor else None
            in_names, out_names, out_avals, zero_outs = [], [], [], []
            for alloc in nc.m.functions[0].allocations:
                if not isinstance(alloc, mybir.MemoryLocationSet):
                    continue
                nm = alloc.memorylocations[0].name
                if alloc.kind == "ExternalInput":
                    if nm != pid_name:
                        in_names.append(nm)
                elif alloc.kind == "ExternalOutput":
                    out_names.append(nm)
                    shape = tuple(alloc.tensor_shape)
                    dtype = mybir.dt.np(alloc.dtype)
                    out_avals.append(jax.core.ShapedArray(shape, dtype))
                    zero_outs.append(np.zeros(shape, dtype))
            n_params = len(in_names)
            all_names = in_names + out_names
            if pid_name is not None:
                all_names = all_names + [pid_name]

            def _body(*args):
                operands = list(args)
                if pid_name is not None:
                    operands.append(bass2jax.partition_id_tensor())
                outs = bass2jax._bass_exec_p.bind(
                    *operands, out_avals=tuple(out_avals), in_names=tuple(all_names),
                    out_names=tuple(out_names), lowering_input_output_aliases=(),
                    sim_require_finite=True, sim_require_nnan=True, nc=nc)
                return tuple(outs)

            devices = jax.devices()[:8]
            mesh = Mesh(np.asarray(devices), ("core",))
            nio = n_params + len(out_names)
            sharded = jax.jit(
                shard_map(_body, mesh=mesh, in_specs=(PartitionSpec("core"),) * nio,
                          out_specs=(PartitionSpec("core"),) * len(out_names),
                          check_rep=False),
                keep_unused=True)
            _jit_cache = (sharded, in_names, out_names, zero_outs)

        sharded, in_names, out_names, zero_outs = _jit_cache
        concat_in = [np.concatenate([np.asarray(m[nm]) for m in in_maps], axis=0)
                     for nm in in_names]
        concat_zero = [np.concatenate([z] * 8, axis=0) for z in zero_outs]
        outs = sharded(*concat_in, *concat_zero)
        results = []
        for c in range(8):
            d = {}
            for i, nm in enumerate(out_names):
                arr = np.asarray(outs[i])
                n0 = arr.shape[0] // 8
                d[nm] = arr[c * n0 : (c + 1) * n0]
            results.append(d)
        return results
    except Exception:
        res = run_bass_kernel_spmd(nc, in_maps, core_ids=list(range(8)))
        return res.results


# revision 43
# speedup vs baseline: 1.0143x; 1.0143x over previous
"""Causal MHA with RoPE on 8 Trainium2 NeuronCores.

Sharding: core c -> batch b=c//2, head-group g=c%2 (8 heads of 16).
Each core: Q/K/V projections for its 512 head-dims over the full sequence,
causal attention for its 8 heads, partial output projection (its 512 rows
of wo). Host sums the two partial outputs per batch. No collectives.

All operands bf16 (fp32 PSUM accumulation), prepared host-side:
 - x^T materialized by XBAR DMA-transpose straight into SBUF (no PE work).
 - Weights/tables loaded once, DMA order latency-tuned (the scheduler
   chains coarsened waits between nearby DMAs, so transfer sizes are kept
   small and ordered by first use).
 - RoPE: dst = C*acc + PM@(S*acc), PM a 32-row block-swap permutation
   matrix as a PE matmul (no SBUF swap DMAs); sign of S folded host-side;
   each chunk's rope tail is emitted after the next chunk's matmuls so the
   PE never waits on it.
 - Causal mask: exp first (ScalarE, scale=1/8 folded in), then one bf16
   DVE multiply of the diagonal 128-tile by a 0/1 lower-triangular mask.
 - AV computed transposed: exp-block stationary, [V | 1] moving ->
   ctx^T [q, dim] at 65 cols per (tile, head) instead of streaming exp
   twice; the ones column yields softmax denominators for free. Each
   (pair, query-chunk) accumulation group is contiguous and owns its PSUM
   tile: interleaved groups within one tile corrupt on hardware.
 - ctx^T scaled by 1/denom (per-partition scalar), transposed back to
   [dim, tok] by XBAR DMA for the output projection; bf16 output summed
   across head-group cores on the host.
Issue order interleaves projections of pass t+1 and the output projection
into the attention stream of pass t (weighted toward the exp-bound prefix
iterations) so ScalarE exp time hides behind PE work.
Timeline-sim: 245973 ns/core (baseline 413016); rel err vs fp32 ref 3.7e-3.
"""
import math
import os

import numpy as np

import concourse.bass as bass
import concourse.mybir as mybir
import concourse.tile as tile
from concourse import bacc
from concourse.bass_utils import run_bass_kernel_spmd

F32 = mybir.dt.float32
BF16 = mybir.dt.bfloat16

B, S, D, H = 4, 2048, 1024, 16
HD = D // H          # 64
THETA = 10000.0
DH = D // 2          # 512 per-core head dims (8 heads)
NP = 4               # head pairs per core
NTH = 4              # token passes
THT = S // NTH       # 512 tokens per pass
QB = THT             # query block
NKT = S // 128       # 16 key tiles of 128
SCALE = 1.0 / math.sqrt(HD)

_cached = None


def _build():
    nc = bacc.Bacc(None, target_bir_lowering=False)

    x = nc.dram_tensor("x", [S, D], BF16, kind="ExternalInput")
    wq = nc.dram_tensor("wq", [128, 8, DH], BF16, kind="ExternalInput")
    wk = nc.dram_tensor("wk", [128, 8, DH], BF16, kind="ExternalInput")
    wv = nc.dram_tensor("wv", [128, 8, DH], BF16, kind="ExternalInput")
    wo = nc.dram_tensor("wo", [128, 4, D], BF16, kind="ExternalInput")
    cosb = nc.dram_tensor("cosb", [128, S], BF16, kind="ExternalInput")
    sinb = nc.dram_tensor("sinb", [128, S], BF16, kind="ExternalInput")
    # [PM | ident | tri01] host-built constants
    consts = nc.dram_tensor("consts", [128, 3, 128], BF16, kind="ExternalInput")
    outp = nc.dram_tensor("outp", [S, D], BF16, kind="ExternalOutput")
    dbg = {}
    if os.environ.get("KDBG"):
        for nm, shp in (("dxt0", [128, S]), ("dkt0", [128, S]), ("dqt0", [128, S]),
                        ("dv0", [128, 8, HD + 1]), ("dctx0", [128, S]),
                        ("deab", [128, 2 * QB]), ("dpse", [128, 4, HD + 1]),
                        ("dctxT", [128, 4, 128])):
            dbg[nm] = nc.dram_tensor(nm, shp, BF16, kind="ExternalOutput")

    with tile.TileContext(nc) as tc:
        with (
            tc.tile_pool(name="const", bufs=1) as cpool,
            tc.tile_pool(name="xt", bufs=1) as xpool,
            tc.tile_pool(name="kq", bufs=1) as kqpool,
            tc.tile_pool(name="vaug", bufs=1) as vpool,
            tc.tile_pool(name="wts", bufs=1) as wpool,
            tc.tile_pool(name="stream", bufs=2) as spool,
        ):
            # weights, loaded once on the Pool queue (parallel to sync queue)
            wq_s = wpool.tile([128, 8, DH], BF16, name="wq_s")
            wk_s = wpool.tile([128, 8, DH], BF16, name="wk_s")
            wv_s = wpool.tile([128, 8, DH], BF16, name="wv_s")
            wo_s = wpool.tile([128, 4, D], BF16, name="wo_s")
            # All loads go through the in-order SP queue: the scheduler's
            # coarsened cross-queue DMA waits serialize arbitrary pairs, so
            # explicit FIFO placement beats a second queue. Weights split in
            # 0.5MB chunks to keep any one hold on the DMA engines short.
            # wk first: K-projection chunks are emitted before Q's.
            def wload(dst, src):
                n = dst.shape[1]
                for c in range(0, n, n // 2):
                    nc.sync.dma_start(out=dst[:, c : c + n // 2, :],
                                      in_=src[:, c : c + n // 2, :])

            ctile = cpool.tile([128, 3, 128], BF16, name="ctile")
            cos_t = cpool.tile([128, S], BF16, name="cos_t")
            sin_t = cpool.tile([128, S], BF16, name="sin_t")
            pmat = ctile[:, 0, :]
            tri01 = ctile[:, 2, :]

            # x^T tiles: xt[dc] = [128 dims, S tokens], via XBAR DMA transpose.
            # The DMA order is latency-tuned: the scheduler adds coarsened
            # waits chaining each DMA to one a few slots earlier (even across
            # queues), so big transfers are interleaved between the x^T
            # chunks in the order compute first needs them.
            xt = [xpool.tile([128, S], BF16, name=f"xt{dc}") for dc in range(8)]

            def xtload(th, dc):
                t0 = th * THT
                nc.sync.dma_start_transpose(
                    out=xt[dc][:, t0 : t0 + THT],
                    in_=x[t0 : t0 + THT, dc * 128 : (dc + 1) * 128],
                )

            def half(dst, src, h):
                nc.sync.dma_start(out=dst[:, h * (S // 2) : (h + 1) * (S // 2)],
                                  in_=src[:, h * (S // 2) : (h + 1) * (S // 2)])

            nc.gpsimd.dma_start(out=ctile, in_=consts[:, :, :])
            half(cos_t, cosb, 0)
            half(sin_t, sinb, 0)
            wload(wk_s, wk)           # wk0, wk1
            for dc in range(8):
                xtload(0, dc)
            half(cos_t, cosb, 1)
            half(sin_t, sinb, 1)
            late_w = {0: (wq_s, wq), 1: (wv_s, wv), 2: (wo_s, wo)}
            for th in range(NTH):
                if th > 0:
                    for dc in range(8):
                        xtload(th, dc)
                if th in late_w:
                    dst, src = late_w[th]
                    wload(dst, src)

            # K^T / Q^T pair tiles: [128 dims (head 2p | head 2p+1), S tokens]
            kt_tiles = [kqpool.tile([128, S], BF16, name=f"ktp{p}") for p in range(NP)]
            qt_tiles = [kqpool.tile([128, S], BF16, name=f"qtp{p}") for p in range(NP)]
            ctx_tiles = [kqpool.tile([128, S], BF16, name=f"ctxp{p}") for p in range(NP)]
            # V tiles with ones column: [128 tokens, 8 heads, 64+1]
            v_tiles = [vpool.tile([128, 8, HD + 1], BF16, name=f"vt{t}") for t in range(NKT)]
            for t in range(NKT):
                # ones column via exp(0*x) = 1
                nc.scalar.activation(
                    v_tiles[t][:, :, HD], ctile[:, 0, 0:8],
                    mybir.ActivationFunctionType.Exp, scale=0.0,
                )

            with (
                tc.tile_pool(name="pst", bufs=2, space="PSUM") as pst,
                tc.tile_pool(name="pssc", bufs=2, space="PSUM") as pssc,
                tc.tile_pool(name="psav", bufs=2, space="PSUM") as psav,
            ):
                # ---------- work-item generators ----------
                def proj_chunks(th, defer=False):
                    t0 = th * THT
                    ts = slice(t0, t0 + THT)

                    def qk_mms(wsb, dst, p):
                        # returns the rope-tail closure; caller emits it after
                        # the NEXT chunk's matmuls so the PE never waits on it
                        acc = pst.tile([128, THT], F32, name="acc", tag="tp")
                        for dc in range(8):
                            nc.tensor.matmul(
                                acc, wsb[:, dc, p * 128 : (p + 1) * 128],
                                xt[dc][:, ts],
                                start=(dc == 0), stop=(dc == 7),
                            )
                        acc_sb = spool.tile([128, THT], BF16, name="acc_sb",
                                            tag="accsb", bufs=4)
                        nc.scalar.copy(acc_sb, acc)  # frees the PSUM slot fast
                        sacc = spool.tile([128, THT], BF16, name="sacc",
                                          tag="sacc", bufs=3)
                        nc.vector.tensor_mul(sacc, acc_sb, sin_t[:, ts])

                        def rope_tail():
                            # dst = C*acc + PM@(S*acc), S sign-folded host-side
                            rps = pst.tile([128, THT], F32, name="rps", tag="tp")
                            nc.tensor.matmul(rps, pmat, sacc)
                            t1 = spool.tile([128, THT], BF16, name="t1",
                                            tag="t1", bufs=3)
                            nc.vector.tensor_mul(t1, acc_sb, cos_t[:, ts])
                            nc.vector.tensor_add(dst[p][:, ts], t1, rps)
                        return rope_tail

                    chunks = []
                    late = []
                    tail_box = [None]

                    def mk_qk(wsb, dst, p):
                        def qk_chunk(wsb=wsb, dst=dst, p=p, tail_box=tail_box):
                            prev = tail_box[0]
                            tail_box[0] = qk_mms(wsb, dst, p)
                            if prev is not None:
                                prev()
                        return qk_chunk

                    if not defer:
                        for wsb, dst in ((wk_s, kt_tiles), (wq_s, qt_tiles)):
                            for p in range(NP):
                                chunks.append(mk_qk(wsb, dst, p))
                    else:
                        # early pairs stay in this phase; late pairs' K/Q
                        # become filler for the exp-bound next phase (they
                        # are only needed by that phase's late units)
                        for p in (0, 1):
                            chunks.append(mk_qk(wk_s, kt_tiles, p))
                            chunks.append(mk_qk(wq_s, qt_tiles, p))
                        for p in (2, 3):
                            late.append(mk_qk(wk_s, kt_tiles, p))
                            late.append(mk_qk(wq_s, qt_tiles, p))

                        def tail_flush(tail_box=tail_box):
                            prev = tail_box[0]
                            tail_box[0] = None
                            if prev is not None:
                                prev()
                        late.append(tail_flush)
                    for tl in range(THT // 128):
                        def v_chunk(tl=tl, t0=t0, th=th, tail_box=tail_box):
                            acc = pst.tile([128, DH], F32, name="vacc", tag="tp")
                            for dc in range(8):
                                nc.tensor.matmul(
                                    acc, xt[dc][:, t0 + tl * 128 : t0 + (tl + 1) * 128],
                                    wv_s[:, dc, :],
                                    start=(dc == 0), stop=(dc == 7),
                                )
                            prev = tail_box[0]
                            tail_box[0] = None
                            if prev is not None:
                                prev()
                            vt = v_tiles[th * (THT // 128) + tl]
                            nc.vector.tensor_copy(
                                vt[:, :, 0:HD],
                                acc.rearrange("a (h d) -> a h d", h=8),
                            )
                        chunks.append(v_chunk)
                    return (chunks, late) if defer else chunks

                def attn_iters(qb):
                    nk = 4 * qb + 4
                    q0 = qb * QB
                    iters = []

                    def av_group(p, qb, qci, eabs, ctxT_box):
                        # one contiguous accumulation group per (p, qci, head):
                        # the tile framework / PSUM HW mishandles interleaved
                        # groups within one tile, so never interleave them.
                        j = 4 * qb + qci
                        pseq = psav.tile([128, HD + 1], F32, name="pseq", tag="av")
                        psoq = psav.tile([128, HD + 1], F32, name="psoq", tag="av")
                        for kt2 in range(j + 1):
                            dj2 = max(0, kt2 - 4 * qb)
                            e0 = (qci - dj2) * 128
                            eab2 = eabs[kt2]
                            nc.tensor.matmul(
                                pseq, eab2[:, e0 : e0 + 128],
                                v_tiles[kt2][:, 2 * p, :],
                                start=(kt2 == 0), stop=(kt2 == j))
                            nc.tensor.matmul(
                                psoq, eab2[:, QB + e0 : QB + e0 + 128],
                                v_tiles[kt2][:, 2 * p + 1, :],
                                start=(kt2 == 0), stop=(kt2 == j))
                        rec = spool.tile([128, 2], F32, name="rec", tag="rec", bufs=4)
                        nc.vector.reciprocal(rec[:, 0:1], pseq[:, HD : HD + 1])
                        nc.vector.reciprocal(rec[:, 1:2], psoq[:, HD : HD + 1])
                        ctxT = spool.tile([128, 128], BF16, name="ctxT",
                                          tag="ctxT", bufs=8)
                        nc.vector.tensor_scalar_mul(ctxT[:, 0:HD], pseq[:, 0:HD],
                                                    rec[:, 0:1])
                        nc.vector.tensor_scalar_mul(ctxT[:, HD:128], psoq[:, 0:HD],
                                                    rec[:, 1:2])
                        tq = (4 * qb + qci) * 128
                        nc.sync.dma_start_transpose(
                            out=ctx_tiles[p][:, tq : tq + 128], in_=ctxT)

                    for p in range(NP):
                        iters.append([])
                        eabs = {}
                        ctxT_box = [None]
                        for j in range(nk):
                            def kt_iter(p=p, j=j, qb=qb, q0=q0, eabs=eabs,
                                        ctxT_box=ctxT_box):
                                kt = j
                                dj = kt - 4 * qb
                                qoff = 128 * dj if dj > 0 else 0
                                n = QB - qoff
                                ktp, qtp = kt_tiles[p], qt_tiles[p]
                                ksl = slice(kt * 128, (kt + 1) * 128)
                                qsl = slice(q0 + qoff, q0 + QB)
                                psab = pssc.tile([128, 2 * QB], F32, name="psab", tag="sc")
                                nc.tensor.matmul(psab[:, 0:n], ktp[0:64, ksl], qtp[0:64, qsl])
                                nc.tensor.matmul(psab[:, QB : QB + n],
                                                 ktp[64:128, ksl], qtp[64:128, qsl])
                                eab = spool.tile([128, 2 * QB], BF16, name="eab",
                                                 tag="eab", bufs=24)
                                eabs[kt] = eab
                                eview_o = bass.AP(
                                    tensor=eab.tensor, offset=eab.offset,
                                    ap=[eab.ap[0], [QB, 2], [1, n]])
                                eview_i = bass.AP(
                                    tensor=psab.tensor, offset=psab.offset,
                                    ap=[psab.ap[0], [QB, 2], [1, n]])
                                nc.scalar.activation(
                                    eview_o, eview_i,
                                    mybir.ActivationFunctionType.Exp, scale=SCALE)
                                if dj >= 0:
                                    # zero exp above the diagonal of the 128-chunk
                                    mview = bass.AP(
                                        tensor=eab.tensor, offset=eab.offset,
                                        ap=[eab.ap[0], [QB, 2], [1, 128]])
                                    tview = bass.AP(
                                        tensor=tri01.tensor, offset=tri01.offset,
                                        ap=[tri01.ap[0], [0, 2], [1, 128]])
                                    nc.vector.tensor_mul(mview, mview, tview)
                                if dbg and qb == 0 and p == 0 and kt == 0:
                                    nc.sync.dma_start(out=dbg["deab"][:, :], in_=eab[:, :])
                                # AV group one iteration behind: its last eab
                                # is already exp'd, so the PE never waits
                                if j - 1 >= 4 * qb:
                                    av_group(p, qb, j - 1 - 4 * qb, eabs, ctxT_box)
                            nmm = 2 * j if j - 1 >= 4 * qb else 0
                            npe = (2 * (QB - (128 * (j - 4 * qb) if j > 4 * qb else 0))
                                   + nmm * (HD + 1)) * 4.167e-4
                            nact = 2 * (QB - (128 * (j - 4 * qb) if j > 4 * qb else 0)) \
                                * 8.33e-4 + 0.37
                            iters[-1].append((kt_iter, npe, nact))

                        def p_flush(p=p, qb=qb, eabs=eabs, ctxT_box=ctxT_box):
                            av_group(p, qb, 3, eabs, ctxT_box)
                        iters[-1].append((p_flush, 2 * nk * (HD + 1) * 4.167e-4, 0.0))
                    return iters

                def oproj_chunk(t, nn):
                    def o_chunk(t=t, nn=nn):
                        oacc = pst.tile([128, 512], F32, name="oacc", tag="tp")
                        for pc in range(4):
                            nc.tensor.matmul(
                                oacc, ctx_tiles[pc][:, t * 128 : (t + 1) * 128],
                                wo_s[:, pc, nn * 512 : (nn + 1) * 512],
                                start=(pc == 0), stop=(pc == 3))
                        osb = spool.tile([128, 512], BF16, name="osb",
                                         tag="osb", bufs=6)
                        nc.vector.tensor_copy(osb, oacc)
                        nc.sync.dma_start(
                            out=outp[t * 128 : (t + 1) * 128,
                                     nn * 512 : (nn + 1) * 512], in_=osb)
                    return o_chunk

                # ---------- schedule ----------
                # qb=2/qb=3 attention units interleave so the exp-heavy tail
                # shares a region with enough PE filler; proj(th) chunks are
                # deadline-scheduled before attn(th) starts, out-proj tiles
                # gated on their query block's last pair finishing.
                def interleave(primary, filler):
                    # spread filler proportionally to each item's PE slack
                    wts = [1.0 if a - p_ > 0.5 else 0.25 for _, p_, a in primary]
                    total = sum(wts)
                    nf = len(filler)
                    fi, acc = 0, 0.0
                    for (fn, _, _), w in zip(primary, wts):
                        fn()
                        acc += w
                        while fi < min(int(acc / total * nf), nf):
                            filler[fi]()
                            fi += 1
                    while fi < nf:
                        filler[fi]()
                        fi += 1

                for c in proj_chunks(0):
                    c()
                aitems = {qb: attn_iters(qb) for qb in range(4)}

                def flat(qb):
                    return [it for pl in aitems[qb] for it in pl]

                for th in (1, 2):
                    interleave(flat(th - 1), proj_chunks(th))
                p3_early, p3_late = proj_chunks(3, defer=True)
                interleave(flat(2), p3_early)
                interleave(flat(3),
                           p3_late + [oproj_chunk(t, nn)
                                      for t in range(12) for nn in (0, 1)])
                for t in range(12, 16):
                    for nn in (0, 1):
                        oproj_chunk(t, nn)()

                if dbg:
                    nc.sync.dma_start(out=dbg["dxt0"][:, :], in_=xt[0][:, :])
                    nc.sync.dma_start(out=dbg["dkt0"][:, :], in_=kt_tiles[0][:, :])
                    nc.sync.dma_start(out=dbg["dqt0"][:, :], in_=qt_tiles[0][:, :])
                    nc.sync.dma_start(out=dbg["dv0"][:, :, :], in_=v_tiles[0][:, :, :])
                    nc.sync.dma_start(out=dbg["dctx0"][:, :], in_=ctx_tiles[0][:, :])

    nc.compile()
    return nc


def _host_tables(token_positions):
    pos = np.asarray(token_positions, dtype=np.float64)
    inv_freq = np.exp(np.arange(0, HD, 2, dtype=np.float64) * (-math.log(THETA) / HD))
    ang = pos[:, None] * inv_freq[None, :]  # [S, 32]
    cos = np.cos(ang).T  # [32, S]
    sin = np.sin(ang).T
    # pair-tile row layout: [head_even: 32 evens | 32 odds][head_odd: same]
    # sign folded so rope = C*acc + PM@(Sx*acc)
    C = np.empty((128, S), np.float64)
    Sx = np.empty((128, S), np.float64)
    for half in range(2):
        r0 = 64 * half
        C[r0 : r0 + 32] = cos
        C[r0 + 32 : r0 + 64] = cos
        Sx[r0 : r0 + 32] = sin
        Sx[r0 + 32 : r0 + 64] = -sin
    return C, Sx


def _host_consts():
    pm = np.zeros((128, 128), np.float64)
    for i in range(128):
        pm[i, i ^ 32] = 1.0
    ident = np.eye(128)
    tri = (np.arange(128)[None, :] >= np.arange(128)[:, None]).astype(np.float64)
    return np.stack([pm, ident, tri], axis=1)  # [128, 3, 128]


def kernel(in_features, token_positions, wq, wk, wv, wo):
    global _cached
    if _cached is None:
        _cached = _build()
    nc = _cached

    from ml_dtypes import bfloat16

    x = np.asarray(in_features, dtype=np.float32)
    # permute wq/wk columns within each head: [evens | odds]
    perm = np.concatenate(
        [64 * h + np.concatenate([np.arange(0, 64, 2), np.arange(1, 64, 2)]) for h in range(H)])
    wqp = np.asarray(wq, np.float32)[:, perm]
    wkp = np.asarray(wk, np.float32)[:, perm]
    wv = np.asarray(wv, np.float32)
    wo = np.asarray(wo, np.float32)
    C, Sx = _host_tables(token_positions)
    consts = _host_consts().astype(bfloat16)
    Cb = C.astype(bfloat16)
    Sb = Sx.astype(bfloat16)

    def wlayout(w):  # [1024, 512] -> [128, 8, 512] chunk-major
        return np.ascontiguousarray(
            w.reshape(8, 128, DH).transpose(1, 0, 2).astype(bfloat16))

    in_maps = []
    for c in range(8):
        b, g = c // 2, c % 2
        sl = slice(g * DH, (g + 1) * DH)
        wo_core = wo[sl, :]  # [512, 1024]
        in_maps.append({
            "x": np.ascontiguousarray(x[b].astype(bfloat16)),
            "wq": wlayout(wqp[:, sl]),
            "wk": wlayout(wkp[:, sl]),
            "wv": wlayout(wv[:, sl]),
            "wo": np.ascontiguousarray(
                wo_core.reshape(4, 128, D).transpose(1, 0, 2).astype(bfloat16)),
            "cosb": Cb,
            "sinb": Sb,
            "consts": consts,
        })
    results = _run(nc, in_maps)
    out = np.empty((B, S, D), np.float32)
    for b in range(B):
        out[b] = (results[2 * b]["outp"].astype(np.float32)
                  + results[2 * b + 1]["outp"].astype(np.float32))
    return out


_jit_cache = None


def _run(nc, in_maps):
    """Run the SPMD program on 8 cores, caching the jitted executable across
    calls (run_bass_kernel_spmd retraces every call). Falls back to the
    library path on any failure."""
    global _jit_cache
    try:
        import jax
        from jax.sharding import Mesh, PartitionSpec
        from jax.experimental.shard_map import shard_map
        from concourse import bass2jax
        import concourse.mybir as mybir

        if _jit_cache is None:
            bass2jax.install_neuronx_cc_hook()
            pid_name = nc.partition_id_tensor.name if nc.partition_id_tensor else None
            in_names, out_names, out_avals, zero_outs = [], [], [], []
            for alloc in nc.m.functions[0].allocations:
                if not isinstance(alloc, mybir.MemoryLocationSet):
                    continue
                nm = alloc.memorylocations[0].name
                if alloc.kind == "ExternalInput":
                    if nm != pid_name:
                        in_names.append(nm)
                elif alloc.kind == "ExternalOutput":
                    out_names.append(nm)
                    shape = tuple(alloc.tensor_shape)
                    dtype = mybir.dt.np(alloc.dtype)
                    out_avals.append(jax.core.ShapedArray(shape, dtype))
                    zero_outs.append(np.zeros(shape, dtype))
            n_params = len(in_names)
            all_names = in_names + out_names
            if pid_name is not None:
                all_names = all_names + [pid_name]

            def _body(*args):
                operands = list(args)
                if pid_name is not None:
                    operands.append(bass2jax.partition_id_tensor())
                outs = bass2jax._bass_exec_p.bind(
                    *operands, out_avals=tuple(out_avals), in_names=tuple(all_names),
                    out_names=tuple(out_names), lowering_input_output_aliases=(),
                    sim_require_finite=True, sim_require_nnan=True, nc=nc)
                return tuple(outs)

            devices = jax.devices()[:8]
            mesh = Mesh(np.asarray(devices), ("core",))
            nio = n_params + len(out_names)
            sharded = jax.jit(
                shard_map(_body, mesh=mesh, in_specs=(PartitionSpec("core"),) * nio,
                          out_specs=(PartitionSpec("core"),) * len(out_names),
                          check_rep=False),
                keep_unused=True)
            _jit_cache = (sharded, in_names, out_names, zero_outs)

        sharded, in_names, out_names, zero_outs = _jit_cache
        concat_in = [np.concatenate([np.asarray(m[nm]) for m in in_maps], axis=0)
                     for nm in in_names]
        concat_zero = [np.concatenate([z] * 8, axis=0) for z in zero_outs]
        outs = sharded(*concat_in, *concat_zero)
        results = []
        for c in range(8):
            d = {}
            for i, nm in enumerate(out_names):
                arr = np.asarray(outs[i])
                n0 = arr.shape[0] // 8
                d[nm] = arr[c * n0 : (c + 1) * n0]
            results.append(d)
        return results
    except Exception:
        res = run_bass_kernel_spmd(nc, in_maps, core_ids=list(range(8)))
        return res.results


# revision 44
# speedup vs baseline: 1.0160x; 1.0017x over previous
"""Causal MHA with RoPE on 8 Trainium2 NeuronCores.

Sharding: core c -> batch b=c//2, head-group g=c%2 (8 heads of 16).
Each core: Q/K/V projections for its 512 head-dims over the full sequence,
causal attention for its 8 heads, partial output projection (its 512 rows
of wo). Host sums the two partial outputs per batch. No collectives.

All operands bf16 (fp32 PSUM accumulation), prepared host-side:
 - x^T materialized by XBAR DMA-transpose straight into SBUF (no PE work).
 - Weights/tables loaded once, DMA order latency-tuned (the scheduler
   chains coarsened waits between nearby DMAs, so transfer sizes are kept
   small and ordered by first use).
 - RoPE: dst = C*acc + PM@(S*acc), PM a 32-row block-swap permutation
   matrix as a PE matmul (no SBUF swap DMAs); sign of S folded host-side;
   each chunk's rope tail is emitted after the next chunk's matmuls so the
   PE never waits on it.
 - Causal mask: exp first (ScalarE, scale=1/8 folded in), then one bf16
   DVE multiply of the diagonal 128-tile by a 0/1 lower-triangular mask.
 - AV computed transposed: exp-block stationary, [V | 1] moving ->
   ctx^T [q, dim] at 65 cols per (tile, head) instead of streaming exp
   twice; the ones column yields softmax denominators for free. Each
   (pair, query-chunk) accumulation group is contiguous and owns its PSUM
   tile: interleaved groups within one tile corrupt on hardware.
 - ctx^T scaled by 1/denom (per-partition scalar), transposed back to
   [dim, tok] by XBAR DMA for the output projection; bf16 output summed
   across head-group cores on the host.
Issue order interleaves projections of pass t+1 and the output projection
into the attention stream of pass t (weighted toward the exp-bound prefix
iterations) so ScalarE exp time hides behind PE work.
Timeline-sim: 245973 ns/core (baseline 413016); rel err vs fp32 ref 3.7e-3.
"""
import math
import os

import numpy as np

import concourse.bass as bass
import concourse.mybir as mybir
import concourse.tile as tile
from concourse import bacc
from concourse.bass_utils import run_bass_kernel_spmd

F32 = mybir.dt.float32
BF16 = mybir.dt.bfloat16

B, S, D, H = 4, 2048, 1024, 16
HD = D // H          # 64
THETA = 10000.0
DH = D // 2          # 512 per-core head dims (8 heads)
NP = 4               # head pairs per core
NTH = 4              # token passes
THT = S // NTH       # 512 tokens per pass
QB = THT             # query block
NKT = S // 128       # 16 key tiles of 128
SCALE = 1.0 / math.sqrt(HD)

_cached = None


def _build():
    nc = bacc.Bacc(None, target_bir_lowering=False)

    x = nc.dram_tensor("x", [S, D], BF16, kind="ExternalInput")
    wq = nc.dram_tensor("wq", [128, 8, DH], BF16, kind="ExternalInput")
    wk = nc.dram_tensor("wk", [128, 8, DH], BF16, kind="ExternalInput")
    wv = nc.dram_tensor("wv", [128, 8, DH], BF16, kind="ExternalInput")
    wo = nc.dram_tensor("wo", [128, 4, D], BF16, kind="ExternalInput")
    cosb = nc.dram_tensor("cosb", [128, S], BF16, kind="ExternalInput")
    sinb = nc.dram_tensor("sinb", [128, S], BF16, kind="ExternalInput")
    # [PM | ident | tri01] host-built constants
    consts = nc.dram_tensor("consts", [128, 3, 128], BF16, kind="ExternalInput")
    outp = nc.dram_tensor("outp", [S, D], BF16, kind="ExternalOutput")
    dbg = {}
    if os.environ.get("KDBG"):
        for nm, shp in (("dxt0", [128, S]), ("dkt0", [128, S]), ("dqt0", [128, S]),
                        ("dv0", [128, 8, HD + 1]), ("dctx0", [128, S]),
                        ("deab", [128, 2 * QB]), ("dpse", [128, 4, HD + 1]),
                        ("dctxT", [128, 4, 128])):
            dbg[nm] = nc.dram_tensor(nm, shp, BF16, kind="ExternalOutput")

    with tile.TileContext(nc) as tc:
        with (
            tc.tile_pool(name="const", bufs=1) as cpool,
            tc.tile_pool(name="xt", bufs=1) as xpool,
            tc.tile_pool(name="kq", bufs=1) as kqpool,
            tc.tile_pool(name="vaug", bufs=1) as vpool,
            tc.tile_pool(name="wts", bufs=1) as wpool,
            tc.tile_pool(name="stream", bufs=2) as spool,
        ):
            # weights, loaded once on the Pool queue (parallel to sync queue)
            wq_s = wpool.tile([128, 8, DH], BF16, name="wq_s")
            wk_s = wpool.tile([128, 8, DH], BF16, name="wk_s")
            wv_s = wpool.tile([128, 8, DH], BF16, name="wv_s")
            wo_s = wpool.tile([128, 4, D], BF16, name="wo_s")
            # All loads go through the in-order SP queue: the scheduler's
            # coarsened cross-queue DMA waits serialize arbitrary pairs, so
            # explicit FIFO placement beats a second queue. Weights split in
            # 0.5MB chunks to keep any one hold on the DMA engines short.
            # wk first: K-projection chunks are emitted before Q's.
            def wload(dst, src):
                n = dst.shape[1]
                for c in range(0, n, n // 2):
                    nc.sync.dma_start(out=dst[:, c : c + n // 2, :],
                                      in_=src[:, c : c + n // 2, :])

            ctile = cpool.tile([128, 3, 128], BF16, name="ctile")
            cos_t = cpool.tile([128, S], BF16, name="cos_t")
            sin_t = cpool.tile([128, S], BF16, name="sin_t")
            pmat = ctile[:, 0, :]
            tri01 = ctile[:, 2, :]

            # x^T tiles: xt[dc] = [128 dims, S tokens], via XBAR DMA transpose.
            # The DMA order is latency-tuned: the scheduler adds coarsened
            # waits chaining each DMA to one a few slots earlier (even across
            # queues), so big transfers are interleaved between the x^T
            # chunks in the order compute first needs them.
            xt = [xpool.tile([128, S], BF16, name=f"xt{dc}") for dc in range(8)]

            def xtload(th, dc):
                t0 = th * THT
                nc.sync.dma_start_transpose(
                    out=xt[dc][:, t0 : t0 + THT],
                    in_=x[t0 : t0 + THT, dc * 128 : (dc + 1) * 128],
                )

            def half(dst, src, h):
                nc.sync.dma_start(out=dst[:, h * (S // 2) : (h + 1) * (S // 2)],
                                  in_=src[:, h * (S // 2) : (h + 1) * (S // 2)])

            nc.gpsimd.dma_start(out=ctile, in_=consts[:, :, :])
            half(cos_t, cosb, 0)
            half(sin_t, sinb, 0)
            wload(wk_s, wk)           # wk0, wk1
            for dc in range(8):
                xtload(0, dc)
            half(cos_t, cosb, 1)
            half(sin_t, sinb, 1)
            late_w = {0: (wq_s, wq), 1: (wv_s, wv), 2: (wo_s, wo)}
            for th in range(NTH):
                if th > 0:
                    for dc in range(8):
                        xtload(th, dc)
                if th in late_w:
                    dst, src = late_w[th]
                    wload(dst, src)

            # K^T / Q^T pair tiles: [128 dims (head 2p | head 2p+1), S tokens]
            kt_tiles = [kqpool.tile([128, S], BF16, name=f"ktp{p}") for p in range(NP)]
            qt_tiles = [kqpool.tile([128, S], BF16, name=f"qtp{p}") for p in range(NP)]
            ctx_tiles = [kqpool.tile([128, S], BF16, name=f"ctxp{p}") for p in range(NP)]
            # V tiles with ones column: [128 tokens, 8 heads, 64+1]
            v_tiles = [vpool.tile([128, 8, HD + 1], BF16, name=f"vt{t}") for t in range(NKT)]
            for t in range(NKT):
                # ones column via exp(0*x) = 1
                nc.scalar.activation(
                    v_tiles[t][:, :, HD], ctile[:, 0, 0:8],
                    mybir.ActivationFunctionType.Exp, scale=0.0,
                )

            with (
                tc.tile_pool(name="pst", bufs=2, space="PSUM") as pst,
                tc.tile_pool(name="pssc", bufs=2, space="PSUM") as pssc,
                tc.tile_pool(name="psav", bufs=2, space="PSUM") as psav,
            ):
                # ---------- work-item generators ----------
                def proj_chunks(th, defer=False):
                    t0 = th * THT
                    ts = slice(t0, t0 + THT)

                    def qk_mms(wsb, dst, p):
                        # returns the rope-tail closure; caller emits it after
                        # the NEXT chunk's matmuls so the PE never waits on it
                        acc = pst.tile([128, THT], F32, name="acc", tag="tp")
                        for dc in range(8):
                            nc.tensor.matmul(
                                acc, wsb[:, dc, p * 128 : (p + 1) * 128],
                                xt[dc][:, ts],
                                start=(dc == 0), stop=(dc == 7),
                            )
                        acc_sb = spool.tile([128, THT], BF16, name="acc_sb",
                                            tag="accsb", bufs=4)
                        nc.scalar.copy(acc_sb, acc)  # frees the PSUM slot fast
                        sacc = spool.tile([128, THT], BF16, name="sacc",
                                          tag="sacc", bufs=3)
                        nc.vector.tensor_mul(sacc, acc_sb, sin_t[:, ts])

                        def rope_tail():
                            # dst = C*acc + PM@(S*acc), S sign-folded host-side
                            rps = pst.tile([128, THT], F32, name="rps", tag="tp")
                            nc.tensor.matmul(rps, pmat, sacc)
                            t1 = spool.tile([128, THT], BF16, name="t1",
                                            tag="t1", bufs=3)
                            nc.vector.tensor_mul(t1, acc_sb, cos_t[:, ts])
                            nc.vector.tensor_add(dst[p][:, ts], t1, rps)
                        return rope_tail

                    chunks = []
                    late = []
                    tail_box = [None]

                    def mk_qk(wsb, dst, p):
                        def qk_chunk(wsb=wsb, dst=dst, p=p, tail_box=tail_box):
                            prev = tail_box[0]
                            tail_box[0] = qk_mms(wsb, dst, p)
                            if prev is not None:
                                prev()
                        return qk_chunk

                    if not defer:
                        for wsb, dst in ((wk_s, kt_tiles), (wq_s, qt_tiles)):
                            for p in range(NP):
                                chunks.append(mk_qk(wsb, dst, p))
                    else:
                        # early pairs stay in this phase; late pairs' K/Q
                        # become filler for the exp-bound next phase (they
                        # are only needed by that phase's late units)
                        for p in (0, 1):
                            chunks.append(mk_qk(wk_s, kt_tiles, p))
                            chunks.append(mk_qk(wq_s, qt_tiles, p))
                        for p in (2, 3):
                            late.append(mk_qk(wk_s, kt_tiles, p))
                            late.append(mk_qk(wq_s, qt_tiles, p))

                        def tail_flush(tail_box=tail_box):
                            prev = tail_box[0]
                            tail_box[0] = None
                            if prev is not None:
                                prev()
                        late.append(tail_flush)
                    for tl in range(THT // 128):
                        def v_chunk(tl=tl, t0=t0, th=th, tail_box=tail_box):
                            acc = pst.tile([128, DH], F32, name="vacc", tag="tp")
                            for dc in range(8):
                                nc.tensor.matmul(
                                    acc, xt[dc][:, t0 + tl * 128 : t0 + (tl + 1) * 128],
                                    wv_s[:, dc, :],
                                    start=(dc == 0), stop=(dc == 7),
                                )
                            prev = tail_box[0]
                            tail_box[0] = None
                            if prev is not None:
                                prev()
                            vt = v_tiles[th * (THT // 128) + tl]
                            nc.vector.tensor_copy(
                                vt[:, :, 0:HD],
                                acc.rearrange("a (h d) -> a h d", h=8),
                            )
                        chunks.append(v_chunk)
                    return (chunks, late) if defer else chunks

                def attn_iters(qb):
                    nk = 4 * qb + 4
                    q0 = qb * QB
                    iters = []

                    def av_group(p, qb, qci, eabs, ctxT_box):
                        # one contiguous accumulation group per (p, qci, head):
                        # the tile framework / PSUM HW mishandles interleaved
                        # groups within one tile, so never interleave them.
                        j = 4 * qb + qci
                        pseq = psav.tile([128, HD + 1], F32, name="pseq", tag="av")
                        psoq = psav.tile([128, HD + 1], F32, name="psoq", tag="av")
                        for kt2 in range(j + 1):
                            dj2 = max(0, kt2 - 4 * qb)
                            e0 = (qci - dj2) * 128
                            eab2 = eabs[kt2]
                            nc.tensor.matmul(
                                pseq, eab2[:, e0 : e0 + 128],
                                v_tiles[kt2][:, 2 * p, :],
                                start=(kt2 == 0), stop=(kt2 == j))
                            nc.tensor.matmul(
                                psoq, eab2[:, QB + e0 : QB + e0 + 128],
                                v_tiles[kt2][:, 2 * p + 1, :],
                                start=(kt2 == 0), stop=(kt2 == j))
                        rec = spool.tile([128, 2], F32, name="rec", tag="rec", bufs=4)
                        nc.vector.reciprocal(rec[:, 0:1], pseq[:, HD : HD + 1])
                        nc.vector.reciprocal(rec[:, 1:2], psoq[:, HD : HD + 1])
                        ctxT = spool.tile([128, 128], BF16, name="ctxT",
                                          tag="ctxT", bufs=8)
                        nc.vector.tensor_scalar_mul(ctxT[:, 0:HD], pseq[:, 0:HD],
                                                    rec[:, 0:1])
                        nc.vector.tensor_scalar_mul(ctxT[:, HD:128], psoq[:, 0:HD],
                                                    rec[:, 1:2])
                        tq = (4 * qb + qci) * 128
                        nc.sync.dma_start_transpose(
                            out=ctx_tiles[p][:, tq : tq + 128], in_=ctxT)

                    for p in range(NP):
                        iters.append([])
                        eabs = {}
                        ctxT_box = [None]
                        for j in range(nk):
                            def kt_iter(p=p, j=j, qb=qb, q0=q0, eabs=eabs,
                                        ctxT_box=ctxT_box):
                                kt = j
                                dj = kt - 4 * qb
                                qoff = 128 * dj if dj > 0 else 0
                                n = QB - qoff
                                ktp, qtp = kt_tiles[p], qt_tiles[p]
                                ksl = slice(kt * 128, (kt + 1) * 128)
                                qsl = slice(q0 + qoff, q0 + QB)
                                psab = pssc.tile([128, 2 * QB], F32, name="psab", tag="sc")
                                nc.tensor.matmul(psab[:, 0:n], ktp[0:64, ksl], qtp[0:64, qsl])
                                nc.tensor.matmul(psab[:, QB : QB + n],
                                                 ktp[64:128, ksl], qtp[64:128, qsl])
                                eab = spool.tile([128, 2 * QB], BF16, name="eab",
                                                 tag="eab", bufs=24)
                                eabs[kt] = eab
                                eview_o = bass.AP(
                                    tensor=eab.tensor, offset=eab.offset,
                                    ap=[eab.ap[0], [QB, 2], [1, n]])
                                eview_i = bass.AP(
                                    tensor=psab.tensor, offset=psab.offset,
                                    ap=[psab.ap[0], [QB, 2], [1, n]])
                                nc.scalar.activation(
                                    eview_o, eview_i,
                                    mybir.ActivationFunctionType.Exp, scale=SCALE)
                                if dj >= 0:
                                    # zero exp above the diagonal of the 128-chunk
                                    mview = bass.AP(
                                        tensor=eab.tensor, offset=eab.offset,
                                        ap=[eab.ap[0], [QB, 2], [1, 128]])
                                    tview = bass.AP(
                                        tensor=tri01.tensor, offset=tri01.offset,
                                        ap=[tri01.ap[0], [0, 2], [1, 128]])
                                    nc.vector.tensor_mul(mview, mview, tview)
                                if dbg and qb == 0 and p == 0 and kt == 0:
                                    nc.sync.dma_start(out=dbg["deab"][:, :], in_=eab[:, :])
                                # AV group one iteration behind: its last eab
                                # is already exp'd, so the PE never waits
                                if j - 1 >= 4 * qb:
                                    av_group(p, qb, j - 1 - 4 * qb, eabs, ctxT_box)
                            nmm = 2 * j if j - 1 >= 4 * qb else 0
                            npe = (2 * (QB - (128 * (j - 4 * qb) if j > 4 * qb else 0))
                                   + nmm * (HD + 1)) * 4.167e-4
                            nact = 2 * (QB - (128 * (j - 4 * qb) if j > 4 * qb else 0)) \
                                * 8.33e-4 + 0.37
                            iters[-1].append((kt_iter, npe, nact))

                        def p_flush(p=p, qb=qb, eabs=eabs, ctxT_box=ctxT_box):
                            av_group(p, qb, 3, eabs, ctxT_box)
                        iters[-1].append((p_flush, 2 * nk * (HD + 1) * 4.167e-4, 0.0))
                    return iters

                def oproj_chunk(t, nn):
                    def o_chunk(t=t, nn=nn):
                        oacc = pst.tile([128, 512], F32, name="oacc", tag="tp")
                        for pc in range(4):
                            nc.tensor.matmul(
                                oacc, ctx_tiles[pc][:, t * 128 : (t + 1) * 128],
                                wo_s[:, pc, nn * 512 : (nn + 1) * 512],
                                start=(pc == 0), stop=(pc == 3))
                        osb = spool.tile([128, 512], BF16, name="osb",
                                         tag="osb", bufs=6)
                        nc.vector.tensor_copy(osb, oacc)
                        nc.sync.dma_start(
                            out=outp[t * 128 : (t + 1) * 128,
                                     nn * 512 : (nn + 1) * 512], in_=osb)
                    return o_chunk

                # ---------- schedule ----------
                # qb=2/qb=3 attention units interleave so the exp-heavy tail
                # shares a region with enough PE filler; proj(th) chunks are
                # deadline-scheduled before attn(th) starts, out-proj tiles
                # gated on their query block's last pair finishing.
                def interleave(primary, filler):
                    # spread filler proportionally to each item's PE slack
                    wts = [1.0 if a - p_ > 0.5 else 0.25 for _, p_, a in primary]
                    total = sum(wts)
                    nf = len(filler)
                    fi, acc = 0, 0.0
                    for (fn, _, _), w in zip(primary, wts):
                        fn()
                        acc += w
                        while fi < min(int(acc / total * nf), nf):
                            filler[fi]()
                            fi += 1
                    while fi < nf:
                        filler[fi]()
                        fi += 1

                for c in proj_chunks(0):
                    c()
                aitems = {qb: attn_iters(qb) for qb in range(4)}

                def flat(qb):
                    return [it for pl in aitems[qb] for it in pl]

                interleave(flat(0), proj_chunks(1))
                p2_early, p2_late = proj_chunks(2, defer=True)
                interleave(flat(1), p2_early)
                p3_early, p3_late = proj_chunks(3, defer=True)
                interleave(flat(2), p2_late + p3_early)
                interleave(flat(3),
                           p3_late + [oproj_chunk(t, nn)
                                      for t in range(12) for nn in (0, 1)])
                for t in range(12, 16):
                    for nn in (0, 1):
                        oproj_chunk(t, nn)()

                if dbg:
                    nc.sync.dma_start(out=dbg["dxt0"][:, :], in_=xt[0][:, :])
                    nc.sync.dma_start(out=dbg["dkt0"][:, :], in_=kt_tiles[0][:, :])
                    nc.sync.dma_start(out=dbg["dqt0"][:, :], in_=qt_tiles[0][:, :])
                    nc.sync.dma_start(out=dbg["dv0"][:, :, :], in_=v_tiles[0][:, :, :])
                    nc.sync.dma_start(out=dbg["dctx0"][:, :], in_=ctx_tiles[0][:, :])

    nc.compile()
    return nc


def _host_tables(token_positions):
    pos = np.asarray(token_positions, dtype=np.float64)
    inv_freq = np.exp(np.arange(0, HD, 2, dtype=np.float64) * (-math.log(THETA) / HD))
    ang = pos[:, None] * inv_freq[None, :]  # [S, 32]
    cos = np.cos(ang).T  # [32, S]
    sin = np.sin(ang).T
    # pair-tile row layout: [head_even: 32 evens | 32 odds][head_odd: same]
    # sign folded so rope = C*acc + PM@(Sx*acc)
    C = np.empty((128, S), np.float64)
    Sx = np.empty((128, S), np.float64)
    for half in range(2):
        r0 = 64 * half
        C[r0 : r0 + 32] = cos
        C[r0 + 32 : r0 + 64] = cos
        Sx[r0 : r0 + 32] = sin
        Sx[r0 + 32 : r0 + 64] = -sin
    return C, Sx


def _host_consts():
    pm = np.zeros((128, 128), np.float64)
    for i in range(128):
        pm[i, i ^ 32] = 1.0
    ident = np.eye(128)
    tri = (np.arange(128)[None, :] >= np.arange(128)[:, None]).astype(np.float64)
    return np.stack([pm, ident, tri], axis=1)  # [128, 3, 128]


def kernel(in_features, token_positions, wq, wk, wv, wo):
    global _cached
    if _cached is None:
        _cached = _build()
    nc = _cached

    from ml_dtypes import bfloat16

    x = np.asarray(in_features, dtype=np.float32)
    # permute wq/wk columns within each head: [evens | odds]
    perm = np.concatenate(
        [64 * h + np.concatenate([np.arange(0, 64, 2), np.arange(1, 64, 2)]) for h in range(H)])
    wqp = np.asarray(wq, np.float32)[:, perm]
    wkp = np.asarray(wk, np.float32)[:, perm]
    wv = np.asarray(wv, np.float32)
    wo = np.asarray(wo, np.float32)
    C, Sx = _host_tables(token_positions)
    consts = _host_consts().astype(bfloat16)
    Cb = C.astype(bfloat16)
    Sb = Sx.astype(bfloat16)

    def wlayout(w):  # [1024, 512] -> [128, 8, 512] chunk-major
        return np.ascontiguousarray(
            w.reshape(8, 128, DH).transpose(1, 0, 2).astype(bfloat16))

    in_maps = []
    for c in range(8):
        b, g = c // 2, c % 2
        sl = slice(g * DH, (g + 1) * DH)
        wo_core = wo[sl, :]  # [512, 1024]
        in_maps.append({
            "x": np.ascontiguousarray(x[b].astype(bfloat16)),
            "wq": wlayout(wqp[:, sl]),
            "wk": wlayout(wkp[:, sl]),
            "wv": wlayout(wv[:, sl]),
            "wo": np.ascontiguousarray(
                wo_core.reshape(4, 128, D).transpose(1, 0, 2).astype(bfloat16)),
            "cosb": Cb,
            "sinb": Sb,
            "consts": consts,
        })
    results = _run(nc, in_maps)
    out = np.empty((B, S, D), np.float32)
    for b in range(B):
        out[b] = (results[2 * b]["outp"].astype(np.float32)
                  + results[2 * b + 1]["outp"].astype(np.float32))
    return out


_jit_cache = None


def _run(nc, in_maps):
    """Run the SPMD program on 8 cores, caching the jitted executable across
    calls (run_bass_kernel_spmd retraces every call). Falls back to the
    library path on any failure."""
    global _jit_cache
    try:
        import jax
        from jax.sharding import Mesh, PartitionSpec
        from jax.experimental.shard_map import shard_map
        from concourse import bass2jax
        import concourse.mybir as mybir

        if _jit_cache is None:
            bass2jax.install_neuronx_cc_hook()
            pid_name = nc.partition_id_tensor.name if nc.partition_id_tensor else None
            in_names, out_names, out_avals, zero_outs = [], [], [], []
            for alloc in nc.m.functions[0].allocations:
                if not isinstance(alloc, mybir.MemoryLocationSet):
                    continue
                nm = alloc.memorylocations[0].name
                if alloc.kind == "ExternalInput":
                    if nm != pid_name:
                        in_names.append(nm)
                elif alloc.kind == "ExternalOutput":
                    out_names.append(nm)
                    shape = tuple(alloc.tensor_shape)
                    dtype = mybir.dt.np(alloc.dtype)
                    out_avals.append(jax.core.ShapedArray(shape, dtype))
                    zero_outs.append(np.zeros(shape, dtype))
            n_params = len(in_names)
            all_names = in_names + out_names
            if pid_name is not None:
                all_names = all_names + [pid_name]

            def _body(*args):
                operands = list(args)
                if pid_name is not None:
                    operands.append(bass2jax.partition_id_tensor())
                outs = bass2jax._bass_exec_p.bind(
                    *operands, out_avals=tuple(out_avals), in_names=tuple(all_names),
                    out_names=tuple(out_names), lowering_input_output_aliases=(),
                    sim_require_finite=True, sim_require_nnan=True, nc=nc)
                return tuple(outs)

            devices = jax.devices()[:8]
            mesh = Mesh(np.asarray(devices), ("core",))
            nio = n_params + len(out_names)
            sharded = jax.jit(
                shard_map(_body, mesh=mesh, in_specs=(PartitionSpec("core"),) * nio,
                          out_specs=(PartitionSpec("core"),) * len(out_names),
                          check_rep=False),
                keep_unused=True)
            _jit_cache = (sharded, in_names, out_names, zero_outs)

        sharded, in_names, out_names, zero_outs = _jit_cache
        concat_in = [np.concatenate([np.asarray(m[nm]) for m in in_maps], axis=0)
                     for nm in in_names]
        concat_zero = [np.concatenate([z] * 8, axis=0) for z in zero_outs]
        outs = sharded(*concat_in, *concat_zero)
        results = []
        for c in range(8):
            d = {}
            for i, nm in enumerate(out_names):
                arr = np.asarray(outs[i])
                n0 = arr.shape[0] // 8
                d[nm] = arr[c * n0 : (c + 1) * n0]
            results.append(d)
        return results
    except Exception:
        res = run_bass_kernel_spmd(nc, in_maps, core_ids=list(range(8)))
        return res.results


# revision 47
# speedup vs baseline: 1.0203x; 1.0042x over previous
"""Causal MHA with RoPE on 8 Trainium2 NeuronCores.

Sharding: core c -> batch b=c//2, head-group g=c%2 (8 heads of 16).
Each core: Q/K/V projections for its 512 head-dims over the full sequence,
causal attention for its 8 heads, partial output projection (its 512 rows
of wo). Host sums the two partial outputs per batch. No collectives.

All operands bf16 (fp32 PSUM accumulation), prepared host-side:
 - x^T materialized by XBAR DMA-transpose straight into SBUF (no PE work).
 - Weights/tables loaded once, DMA order latency-tuned (the scheduler
   chains coarsened waits between nearby DMAs, so transfer sizes are kept
   small and ordered by first use).
 - RoPE: dst = C*acc + PM@(S*acc), PM a 32-row block-swap permutation
   matrix as a PE matmul (no SBUF swap DMAs); sign of S folded host-side;
   each chunk's rope tail is emitted after the next chunk's matmuls so the
   PE never waits on it.
 - Causal mask: exp first (ScalarE, scale=1/8 folded in), then one bf16
   DVE multiply of the diagonal 128-tile by a 0/1 lower-triangular mask.
 - AV computed transposed: exp-block stationary, [V | 1] moving ->
   ctx^T [q, dim] at 65 cols per (tile, head) instead of streaming exp
   twice; the ones column yields softmax denominators for free. Each
   (pair, query-chunk) accumulation group is contiguous and owns its PSUM
   tile: interleaved groups within one tile corrupt on hardware.
 - ctx^T scaled by 1/denom (per-partition scalar), transposed back to
   [dim, tok] by XBAR DMA for the output projection; bf16 output summed
   across head-group cores on the host.
Issue order interleaves projections of pass t+1 and the output projection
into the attention stream of pass t (weighted toward the exp-bound prefix
iterations) so ScalarE exp time hides behind PE work.
Timeline-sim: 242088 ns/core (baseline 413016, 1.71x); rel err 3.7e-3.
Late projection pairs are deferred into the following attention phase as
extra PE filler for its exp-bound prefix (they are only needed by that
phase's late pairs).
"""
import math
import os

import numpy as np

import concourse.bass as bass
import concourse.mybir as mybir
import concourse.tile as tile
from concourse import bacc
from concourse.bass_utils import run_bass_kernel_spmd

F32 = mybir.dt.float32
BF16 = mybir.dt.bfloat16

B, S, D, H = 4, 2048, 1024, 16
HD = D // H          # 64
THETA = 10000.0
DH = D // 2          # 512 per-core head dims (8 heads)
NP = 4               # head pairs per core
NTH = 4              # token passes
THT = S // NTH       # 512 tokens per pass
QB = THT             # query block
NKT = S // 128       # 16 key tiles of 128
SCALE = 1.0 / math.sqrt(HD)

_cached = None


def _build():
    nc = bacc.Bacc(None, target_bir_lowering=False)

    x = nc.dram_tensor("x", [S, D], BF16, kind="ExternalInput")
    wq = nc.dram_tensor("wq", [128, 8, DH], BF16, kind="ExternalInput")
    wk = nc.dram_tensor("wk", [128, 8, DH], BF16, kind="ExternalInput")
    wv = nc.dram_tensor("wv", [128, 8, DH], BF16, kind="ExternalInput")
    wo = nc.dram_tensor("wo", [128, 4, D], BF16, kind="ExternalInput")
    cosb = nc.dram_tensor("cosb", [128, S], BF16, kind="ExternalInput")
    sinb = nc.dram_tensor("sinb", [128, S], BF16, kind="ExternalInput")
    # [PM | ident | tri01] host-built constants
    consts = nc.dram_tensor("consts", [128, 3, 128], BF16, kind="ExternalInput")
    outp = nc.dram_tensor("outp", [S, D], BF16, kind="ExternalOutput")
    dbg = {}
    if os.environ.get("KDBG"):
        for nm, shp in (("dxt0", [128, S]), ("dkt0", [128, S]), ("dqt0", [128, S]),
                        ("dv0", [128, 8, HD + 1]), ("dctx0", [128, S]),
                        ("deab", [128, 2 * QB]), ("dpse", [128, 4, HD + 1]),
                        ("dctxT", [128, 4, 128])):
            dbg[nm] = nc.dram_tensor(nm, shp, BF16, kind="ExternalOutput")

    with tile.TileContext(nc) as tc:
        with (
            tc.tile_pool(name="const", bufs=1) as cpool,
            tc.tile_pool(name="xt", bufs=1) as xpool,
            tc.tile_pool(name="kq", bufs=1) as kqpool,
            tc.tile_pool(name="vaug", bufs=1) as vpool,
            tc.tile_pool(name="wts", bufs=1) as wpool,
            tc.tile_pool(name="stream", bufs=2) as spool,
        ):
            # weights, loaded once on the Pool queue (parallel to sync queue)
            wq_s = wpool.tile([128, 8, DH], BF16, name="wq_s")
            wk_s = wpool.tile([128, 8, DH], BF16, name="wk_s")
            wv_s = wpool.tile([128, 8, DH], BF16, name="wv_s")
            wo_s = wpool.tile([128, 4, D], BF16, name="wo_s")
            # All loads go through the in-order SP queue: the scheduler's
            # coarsened cross-queue DMA waits serialize arbitrary pairs, so
            # explicit FIFO placement beats a second queue. Weights split in
            # 0.5MB chunks to keep any one hold on the DMA engines short.
            # wk first: K-projection chunks are emitted before Q's.
            def wload(dst, src):
                n = dst.shape[1]
                for c in range(0, n, n // 2):
                    nc.sync.dma_start(out=dst[:, c : c + n // 2, :],
                                      in_=src[:, c : c + n // 2, :])

            ctile = cpool.tile([128, 3, 128], BF16, name="ctile")
            cos_t = cpool.tile([128, S], BF16, name="cos_t")
            sin_t = cpool.tile([128, S], BF16, name="sin_t")
            pmat = ctile[:, 0, :]
            tri01 = ctile[:, 2, :]

            # x^T tiles: xt[dc] = [128 dims, S tokens], via XBAR DMA transpose.
            # The DMA order is latency-tuned: the scheduler adds coarsened
            # waits chaining each DMA to one a few slots earlier (even across
            # queues), so big transfers are interleaved between the x^T
            # chunks in the order compute first needs them.
            xt = [xpool.tile([128, S], BF16, name=f"xt{dc}") for dc in range(8)]

            def xtload(th, dc):
                t0 = th * THT
                nc.sync.dma_start_transpose(
                    out=xt[dc][:, t0 : t0 + THT],
                    in_=x[t0 : t0 + THT, dc * 128 : (dc + 1) * 128],
                )

            def half(dst, src, h):
                nc.sync.dma_start(out=dst[:, h * (S // 2) : (h + 1) * (S // 2)],
                                  in_=src[:, h * (S // 2) : (h + 1) * (S // 2)])

            nc.gpsimd.dma_start(out=ctile, in_=consts[:, :, :])
            half(cos_t, cosb, 0)
            half(sin_t, sinb, 0)
            wload(wk_s, wk)           # wk0, wk1
            for dc in range(8):
                xtload(0, dc)
            half(cos_t, cosb, 1)
            half(sin_t, sinb, 1)
            late_w = {0: (wq_s, wq), 1: (wv_s, wv), 2: (wo_s, wo)}
            for th in range(NTH):
                if th > 0:
                    for dc in range(8):
                        xtload(th, dc)
                if th in late_w:
                    dst, src = late_w[th]
                    wload(dst, src)

            # K^T / Q^T pair tiles: [128 dims (head 2p | head 2p+1), S tokens]
            kt_tiles = [kqpool.tile([128, S], BF16, name=f"ktp{p}") for p in range(NP)]
            qt_tiles = [kqpool.tile([128, S], BF16, name=f"qtp{p}") for p in range(NP)]
            ctx_tiles = [kqpool.tile([128, S], BF16, name=f"ctxp{p}") for p in range(NP)]
            # V tiles with ones column: [128 tokens, 8 heads, 64+1]
            v_tiles = [vpool.tile([128, 8, HD + 1], BF16, name=f"vt{t}") for t in range(NKT)]
            for t in range(NKT):
                # ones column via exp(0*x) = 1
                nc.scalar.activation(
                    v_tiles[t][:, :, HD], ctile[:, 0, 0:8],
                    mybir.ActivationFunctionType.Exp, scale=0.0,
                )

            with (
                tc.tile_pool(name="pst", bufs=2, space="PSUM") as pst,
                tc.tile_pool(name="pssc", bufs=2, space="PSUM") as pssc,
                tc.tile_pool(name="psav", bufs=2, space="PSUM") as psav,
            ):
                # ---------- work-item generators ----------
                def proj_chunks(th, defer=False):
                    t0 = th * THT
                    ts = slice(t0, t0 + THT)

                    def qk_mms(wsb, dst, p, pt):
                        # returns the rope-tail closure; caller emits it after
                        # the NEXT chunk's matmuls so the PE never waits on it
                        pool, tag = pt
                        acc = pool.tile([128, THT], F32, name="acc", tag=tag)
                        for dc in range(8):
                            nc.tensor.matmul(
                                acc, wsb[:, dc, p * 128 : (p + 1) * 128],
                                xt[dc][:, ts],
                                start=(dc == 0), stop=(dc == 7),
                            )
                        acc_sb = spool.tile([128, THT], BF16, name="acc_sb",
                                            tag="accsb", bufs=4)
                        nc.scalar.copy(acc_sb, acc)  # frees the PSUM slot fast
                        sacc = spool.tile([128, THT], BF16, name="sacc",
                                          tag="sacc", bufs=3)
                        nc.vector.tensor_mul(sacc, acc_sb, sin_t[:, ts])

                        def rope_tail():
                            # dst = C*acc + PM@(S*acc), S sign-folded host-side
                            rps = pool.tile([128, THT], F32, name="rps", tag=tag)
                            nc.tensor.matmul(rps, pmat, sacc)
                            t1 = spool.tile([128, THT], BF16, name="t1",
                                            tag="t1", bufs=3)
                            nc.vector.tensor_mul(t1, acc_sb, cos_t[:, ts])
                            nc.vector.tensor_add(dst[p][:, ts], t1, rps)
                        return rope_tail

                    chunks = []
                    late = []
                    tail_box = [None]
                    # in pass 0 the attention PSUM banks are still idle:
                    # alternate accumulators between the two pools so the
                    # ring depth doubles and chunks never wait on evacuation
                    cnt = [0]

                    def next_pt():
                        cnt[0] += 1
                        if th == 0 and cnt[0] <= 6 and cnt[0] % 2 == 0:
                            return (psav, "av")
                        return (pst, "tp")

                    def mk_qk(wsb, dst, p):
                        def qk_chunk(wsb=wsb, dst=dst, p=p, tail_box=tail_box):
                            prev = tail_box[0]
                            tail_box[0] = qk_mms(wsb, dst, p, next_pt())
                            if prev is not None:
                                prev()
                        return qk_chunk

                    if not defer:
                        for wsb, dst in ((wk_s, kt_tiles), (wq_s, qt_tiles)):
                            for p in range(NP):
                                chunks.append(mk_qk(wsb, dst, p))
                    else:
                        # early pairs stay in this phase; late pairs' K/Q
                        # become filler for the exp-bound next phase (they
                        # are only needed by that phase's late units)
                        for p in (0, 1):
                            chunks.append(mk_qk(wk_s, kt_tiles, p))
                            chunks.append(mk_qk(wq_s, qt_tiles, p))
                        for p in (2, 3):
                            late.append(mk_qk(wk_s, kt_tiles, p))
                            late.append(mk_qk(wq_s, qt_tiles, p))

                        def tail_flush(tail_box=tail_box):
                            prev = tail_box[0]
                            tail_box[0] = None
                            if prev is not None:
                                prev()
                        late.append(tail_flush)
                    for tl in range(THT // 128):
                        def v_chunk(tl=tl, t0=t0, th=th, tail_box=tail_box):
                            pool, tag = next_pt()
                            acc = pool.tile([128, DH], F32, name="vacc", tag=tag)
                            for dc in range(8):
                                nc.tensor.matmul(
                                    acc, xt[dc][:, t0 + tl * 128 : t0 + (tl + 1) * 128],
                                    wv_s[:, dc, :],
                                    start=(dc == 0), stop=(dc == 7),
                                )
                            prev = tail_box[0]
                            tail_box[0] = None
                            if prev is not None:
                                prev()
                            vt = v_tiles[th * (THT // 128) + tl]
                            nc.vector.tensor_copy(
                                vt[:, :, 0:HD],
                                acc.rearrange("a (h d) -> a h d", h=8),
                            )
                        chunks.append(v_chunk)
                    return (chunks, late) if defer else chunks

                def attn_iters(qb):
                    nk = 4 * qb + 4
                    q0 = qb * QB
                    iters = []

                    def av_group(p, qb, qci, eabs, ctxT_box):
                        # one contiguous accumulation group per (p, qci, head):
                        # the tile framework / PSUM HW mishandles interleaved
                        # groups within one tile, so never interleave them.
                        j = 4 * qb + qci
                        pseq = psav.tile([128, HD + 1], F32, name="pseq", tag="av")
                        psoq = psav.tile([128, HD + 1], F32, name="psoq", tag="av")
                        for kt2 in range(j + 1):
                            dj2 = max(0, kt2 - 4 * qb)
                            e0 = (qci - dj2) * 128
                            eab2 = eabs[kt2]
                            nc.tensor.matmul(
                                pseq, eab2[:, e0 : e0 + 128],
                                v_tiles[kt2][:, 2 * p, :],
                                start=(kt2 == 0), stop=(kt2 == j))
                            nc.tensor.matmul(
                                psoq, eab2[:, QB + e0 : QB + e0 + 128],
                                v_tiles[kt2][:, 2 * p + 1, :],
                                start=(kt2 == 0), stop=(kt2 == j))
                        rec = spool.tile([128, 2], F32, name="rec", tag="rec", bufs=4)
                        nc.vector.reciprocal(rec[:, 0:1], pseq[:, HD : HD + 1])
                        nc.vector.reciprocal(rec[:, 1:2], psoq[:, HD : HD + 1])
                        ctxT = spool.tile([128, 128], BF16, name="ctxT",
                                          tag="ctxT", bufs=8)
                        nc.vector.tensor_scalar_mul(ctxT[:, 0:HD], pseq[:, 0:HD],
                                                    rec[:, 0:1])
                        nc.vector.tensor_scalar_mul(ctxT[:, HD:128], psoq[:, 0:HD],
                                                    rec[:, 1:2])
                        tq = (4 * qb + qci) * 128
                        nc.sync.dma_start_transpose(
                            out=ctx_tiles[p][:, tq : tq + 128], in_=ctxT)

                    for p in range(NP):
                        iters.append([])
                        eabs = {}
                        ctxT_box = [None]
                        for j in range(nk):
                            def kt_iter(p=p, j=j, qb=qb, q0=q0, eabs=eabs,
                                        ctxT_box=ctxT_box):
                                kt = j
                                dj = kt - 4 * qb
                                qoff = 128 * dj if dj > 0 else 0
                                n = QB - qoff
                                ktp, qtp = kt_tiles[p], qt_tiles[p]
                                ksl = slice(kt * 128, (kt + 1) * 128)
                                qsl = slice(q0 + qoff, q0 + QB)
                                psab = pssc.tile([128, 2 * QB], F32, name="psab", tag="sc")
                                nc.tensor.matmul(psab[:, 0:n], ktp[0:64, ksl], qtp[0:64, qsl])
                                nc.tensor.matmul(psab[:, QB : QB + n],
                                                 ktp[64:128, ksl], qtp[64:128, qsl])
                                eab = spool.tile([128, 2 * QB], BF16, name="eab",
                                                 tag="eab", bufs=24)
                                eabs[kt] = eab
                                eview_o = bass.AP(
                                    tensor=eab.tensor, offset=eab.offset,
                                    ap=[eab.ap[0], [QB, 2], [1, n]])
                                eview_i = bass.AP(
                                    tensor=psab.tensor, offset=psab.offset,
                                    ap=[psab.ap[0], [QB, 2], [1, n]])
                                nc.scalar.activation(
                                    eview_o, eview_i,
                                    mybir.ActivationFunctionType.Exp, scale=SCALE)
                                if dj >= 0:
                                    # zero exp above the diagonal of the 128-chunk
                                    mview = bass.AP(
                                        tensor=eab.tensor, offset=eab.offset,
                                        ap=[eab.ap[0], [QB, 2], [1, 128]])
                                    tview = bass.AP(
                                        tensor=tri01.tensor, offset=tri01.offset,
                                        ap=[tri01.ap[0], [0, 2], [1, 128]])
                                    nc.vector.tensor_mul(mview, mview, tview)
                                if dbg and qb == 0 and p == 0 and kt == 0:
                                    nc.sync.dma_start(out=dbg["deab"][:, :], in_=eab[:, :])
                                # AV group one iteration behind: its last eab
                                # is already exp'd, so the PE never waits
                                if j - 1 >= 4 * qb:
                                    av_group(p, qb, j - 1 - 4 * qb, eabs, ctxT_box)
                            nmm = 2 * j if j - 1 >= 4 * qb else 0
                            npe = (2 * (QB - (128 * (j - 4 * qb) if j > 4 * qb else 0))
                                   + nmm * (HD + 1)) * 4.167e-4
                            nact = 2 * (QB - (128 * (j - 4 * qb) if j > 4 * qb else 0)) \
                                * 8.33e-4 + 0.37
                            iters[-1].append((kt_iter, npe, nact))

                        def p_flush(p=p, qb=qb, eabs=eabs, ctxT_box=ctxT_box):
                            av_group(p, qb, 3, eabs, ctxT_box)
                        iters[-1].append((p_flush, 2 * nk * (HD + 1) * 4.167e-4, 0.0))
                    return iters

                def oproj_chunk(t, nn):
                    def o_chunk(t=t, nn=nn):
                        oacc = pst.tile([128, 512], F32, name="oacc", tag="tp")
                        for pc in range(4):
                            nc.tensor.matmul(
                                oacc, ctx_tiles[pc][:, t * 128 : (t + 1) * 128],
                                wo_s[:, pc, nn * 512 : (nn + 1) * 512],
                                start=(pc == 0), stop=(pc == 3))
                        osb = spool.tile([128, 512], BF16, name="osb",
                                         tag="osb", bufs=6)
                        nc.vector.tensor_copy(osb, oacc)
                        nc.sync.dma_start(
                            out=outp[t * 128 : (t + 1) * 128,
                                     nn * 512 : (nn + 1) * 512], in_=osb)
                    return o_chunk

                # ---------- schedule ----------
                # qb=2/qb=3 attention units interleave so the exp-heavy tail
                # shares a region with enough PE filler; proj(th) chunks are
                # deadline-scheduled before attn(th) starts, out-proj tiles
                # gated on their query block's last pair finishing.
                def interleave(primary, filler):
                    # spread filler proportionally to each item's PE slack
                    wts = [1.0 if a - p_ > 0.5 else 0.25 for _, p_, a in primary]
                    total = sum(wts)
                    nf = len(filler)
                    fi, acc = 0, 0.0
                    for (fn, _, _), w in zip(primary, wts):
                        fn()
                        acc += w
                        while fi < min(int(acc / total * nf), nf):
                            filler[fi]()
                            fi += 1
                    while fi < nf:
                        filler[fi]()
                        fi += 1

                for c in proj_chunks(0):
                    c()
                aitems = {qb: attn_iters(qb) for qb in range(4)}

                def flat(qb):
                    return [it for pl in aitems[qb] for it in pl]

                interleave(flat(0), proj_chunks(1))
                p2_early, p2_late = proj_chunks(2, defer=True)
                interleave(flat(1), p2_early)
                p3_early, p3_late = proj_chunks(3, defer=True)
                interleave(flat(2), p2_late + p3_early)
                interleave(flat(3),
                           p3_late + [oproj_chunk(t, nn)
                                      for t in range(12) for nn in (0, 1)])
                for t in range(12, 16):
                    for nn in (0, 1):
                        oproj_chunk(t, nn)()

                if dbg:
                    nc.sync.dma_start(out=dbg["dxt0"][:, :], in_=xt[0][:, :])
                    nc.sync.dma_start(out=dbg["dkt0"][:, :], in_=kt_tiles[0][:, :])
                    nc.sync.dma_start(out=dbg["dqt0"][:, :], in_=qt_tiles[0][:, :])
                    nc.sync.dma_start(out=dbg["dv0"][:, :, :], in_=v_tiles[0][:, :, :])
                    nc.sync.dma_start(out=dbg["dctx0"][:, :], in_=ctx_tiles[0][:, :])

    nc.compile()
    return nc


def _host_tables(token_positions):
    pos = np.asarray(token_positions, dtype=np.float64)
    inv_freq = np.exp(np.arange(0, HD, 2, dtype=np.float64) * (-math.log(THETA) / HD))
    ang = pos[:, None] * inv_freq[None, :]  # [S, 32]
    cos = np.cos(ang).T  # [32, S]
    sin = np.sin(ang).T
    # pair-tile row layout: [head_even: 32 evens | 32 odds][head_odd: same]
    # sign folded so rope = C*acc + PM@(Sx*acc)
    C = np.empty((128, S), np.float64)
    Sx = np.empty((128, S), np.float64)
    for half in range(2):
        r0 = 64 * half
        C[r0 : r0 + 32] = cos
        C[r0 + 32 : r0 + 64] = cos
        Sx[r0 : r0 + 32] = sin
        Sx[r0 + 32 : r0 + 64] = -sin
    return C, Sx


def _host_consts():
    pm = np.zeros((128, 128), np.float64)
    for i in range(128):
        pm[i, i ^ 32] = 1.0
    ident = np.eye(128)
    tri = (np.arange(128)[None, :] >= np.arange(128)[:, None]).astype(np.float64)
    return np.stack([pm, ident, tri], axis=1)  # [128, 3, 128]


def kernel(in_features, token_positions, wq, wk, wv, wo):
    global _cached
    if _cached is None:
        _cached = _build()
    nc = _cached

    from ml_dtypes import bfloat16

    x = np.asarray(in_features, dtype=np.float32)
    # permute wq/wk columns within each head: [evens | odds]
    perm = np.concatenate(
        [64 * h + np.concatenate([np.arange(0, 64, 2), np.arange(1, 64, 2)]) for h in range(H)])
    wqp = np.asarray(wq, np.float32)[:, perm]
    wkp = np.asarray(wk, np.float32)[:, perm]
    wv = np.asarray(wv, np.float32)
    wo = np.asarray(wo, np.float32)
    C, Sx = _host_tables(token_positions)
    consts = _host_consts().astype(bfloat16)
    Cb = C.astype(bfloat16)
    Sb = Sx.astype(bfloat16)

    def wlayout(w):  # [1024, 512] -> [128, 8, 512] chunk-major
        return np.ascontiguousarray(
            w.reshape(8, 128, DH).transpose(1, 0, 2).astype(bfloat16))

    in_maps = []
    for c in range(8):
        b, g = c // 2, c % 2
        sl = slice(g * DH, (g + 1) * DH)
        wo_core = wo[sl, :]  # [512, 1024]
        in_maps.append({
            "x": np.ascontiguousarray(x[b].astype(bfloat16)),
            "wq": wlayout(wqp[:, sl]),
            "wk": wlayout(wkp[:, sl]),
            "wv": wlayout(wv[:, sl]),
            "wo": np.ascontiguousarray(
                wo_core.reshape(4, 128, D).transpose(1, 0, 2).astype(bfloat16)),
            "cosb": Cb,
            "sinb": Sb,
            "consts": consts,
        })
    results = _run(nc, in_maps)
    out = np.empty((B, S, D), np.float32)
    for b in range(B):
        out[b] = (results[2 * b]["outp"].astype(np.float32)
                  + results[2 * b + 1]["outp"].astype(np.float32))
    return out


_jit_cache = None


def _run(nc, in_maps):
    """Run the SPMD program on 8 cores, caching the jitted executable across
    calls (run_bass_kernel_spmd retraces every call). Falls back to the
    library path on any failure."""
    global _jit_cache
    try:
        import jax
        from jax.sharding import Mesh, PartitionSpec
        from jax.experimental.shard_map import shard_map
        from concourse import bass2jax
        import concourse.mybir as mybir

        if _jit_cache is None:
            bass2jax.install_neuronx_cc_hook()
            pid_name = nc.partition_id_tensor.name if nc.partition_id_tensor else None
            in_names, out_names, out_avals, zero_outs = [], [], [], []
            for alloc in nc.m.functions[0].allocations:
                if not isinstance(alloc, mybir.MemoryLocationSet):
                    continue
                nm = alloc.memorylocations[0].name
                if alloc.kind == "ExternalInput":
                    if nm != pid_name:
                        in_names.append(nm)
                elif alloc.kind == "ExternalOutput":
                    out_names.append(nm)
                    shape = tuple(alloc.tensor_shape)
                    dtype = mybir.dt.np(alloc.dtype)
                    out_avals.append(jax.core.ShapedArray(shape, dtype))
                    zero_outs.append(np.zeros(shape, dtype))
            n_params = len(in_names)
            all_names = in_names + out_names
            if pid_name is not None:
                all_names = all_names + [pid_name]

            def _body(*args):
                operands = list(args)
                if pid_name is not None:
                    operands.append(bass2jax.partition_id_tensor())
                outs = bass2jax._bass_exec_p.bind(
                    *operands, out_avals=tuple(out_avals), in_names=tuple(all_names),
                    out_names=tuple(out_names), lowering_input_output_aliases=(),
                    sim_require_finite=True, sim_require_nnan=True, nc=nc)
                return tuple(outs)

            devices = jax.devices()[:8]
            mesh = Mesh(np.asarray(devices), ("core",))
            nio = n_params + len(out_names)
            sharded = jax.jit(
                shard_map(_body, mesh=mesh, in_specs=(PartitionSpec("core"),) * nio,
                          out_specs=(PartitionSpec("core"),) * len(out_names),
                          check_rep=False),
                keep_unused=True)
            _jit_cache = (sharded, in_names, out_names, zero_outs)

        sharded, in_names, out_names, zero_outs = _jit_cache
        concat_in = [np.concatenate([np.asarray(m[nm]) for m in in_maps], axis=0)
                     for nm in in_names]
        concat_zero = [np.concatenate([z] * 8, axis=0) for z in zero_outs]
        outs = sharded(*concat_in, *concat_zero)
        results = []
        for c in range(8):
            d = {}
            for i, nm in enumerate(out_names):
                arr = np.asarray(outs[i])
                n0 = arr.shape[0] // 8
                d[nm] = arr[c * n0 : (c + 1) * n0]
            results.append(d)
        return results
    except Exception:
        res = run_bass_kernel_spmd(nc, in_maps, core_ids=list(range(8)))
        return res.results


# revision 48
# speedup vs baseline: 1.0391x; 1.0184x over previous
"""Causal MHA with RoPE on 8 Trainium2 NeuronCores.

Sharding: core c -> batch b=c//2, head-group g=c%2 (8 heads of 16).
Each core: Q/K/V projections for its 512 head-dims over the full sequence,
causal attention for its 8 heads, partial output projection (its 512 rows
of wo). Host sums the two partial outputs per batch. No collectives.

All operands bf16 (fp32 PSUM accumulation), prepared host-side:
 - x^T materialized by XBAR DMA-transpose straight into SBUF (no PE work).
 - Weights/tables loaded once, DMA order latency-tuned (the scheduler
   chains coarsened waits between nearby DMAs, so transfer sizes are kept
   small and ordered by first use).
 - RoPE: dst = C*acc + PM@(S*acc), PM a 32-row block-swap permutation
   matrix as a PE matmul (no SBUF swap DMAs); sign of S folded host-side;
   each chunk's rope tail is emitted after the next chunk's matmuls so the
   PE never waits on it.
 - Causal mask: exp first (ScalarE, scale=1/8 folded in), then one bf16
   DVE multiply of the diagonal 128-tile by a 0/1 lower-triangular mask.
 - AV computed transposed: exp-block stationary, [V | 1] moving ->
   ctx^T [q, dim] at 65 cols per (tile, head) instead of streaming exp
   twice; the ones column yields softmax denominators for free. Each
   (pair, query-chunk) accumulation group is contiguous and owns its PSUM
   tile: interleaved groups within one tile corrupt on hardware.
 - ctx^T scaled by 1/denom (per-partition scalar), transposed back to
   [dim, tok] by XBAR DMA for the output projection; bf16 output summed
   across head-group cores on the host.
Issue order interleaves projections of pass t+1 and the output projection
into the attention stream of pass t (weighted toward the exp-bound prefix
iterations) so ScalarE exp time hides behind PE work.
Timeline-sim: 242088 ns/core (baseline 413016, 1.71x); rel err 3.7e-3.
Late projection pairs are deferred into the following attention phase as
extra PE filler for its exp-bound prefix (they are only needed by that
phase's late pairs).
"""
import math
import os

import numpy as np

import concourse.bass as bass
import concourse.mybir as mybir
import concourse.tile as tile
from concourse import bacc
from concourse.bass_utils import run_bass_kernel_spmd

F32 = mybir.dt.float32
BF16 = mybir.dt.bfloat16

B, S, D, H = 4, 2048, 1024, 16
HD = D // H          # 64
THETA = 10000.0
DH = D // 2          # 512 per-core head dims (8 heads)
NP = 4               # head pairs per core
NTH = 4              # token passes
THT = S // NTH       # 512 tokens per pass
QB = THT             # query block
NKT = S // 128       # 16 key tiles of 128
SCALE = 1.0 / math.sqrt(HD)

_cached = None


def _build():
    nc = bacc.Bacc(None, target_bir_lowering=False)

    x = nc.dram_tensor("x", [S, D], BF16, kind="ExternalInput")
    wq = nc.dram_tensor("wq", [128, 8, DH], BF16, kind="ExternalInput")
    wk = nc.dram_tensor("wk", [128, 8, DH], BF16, kind="ExternalInput")
    wv = nc.dram_tensor("wv", [128, 8, DH], BF16, kind="ExternalInput")
    wo = nc.dram_tensor("wo", [128, 4, D], BF16, kind="ExternalInput")
    cosb = nc.dram_tensor("cosb", [128, S], BF16, kind="ExternalInput")
    sinb = nc.dram_tensor("sinb", [128, S], BF16, kind="ExternalInput")
    # [PM | ident | tri01] host-built constants
    consts = nc.dram_tensor("consts", [128, 3, 128], BF16, kind="ExternalInput")
    outp = nc.dram_tensor("outp", [S, D], BF16, kind="ExternalOutput")
    dbg = {}
    if os.environ.get("KDBG"):
        for nm, shp in (("dxt0", [128, S]), ("dkt0", [128, S]), ("dqt0", [128, S]),
                        ("dv0", [128, 8, HD + 1]), ("dctx0", [128, S]),
                        ("deab", [128, 2 * QB]), ("dpse", [128, 4, HD + 1]),
                        ("dctxT", [128, 4, 128])):
            dbg[nm] = nc.dram_tensor(nm, shp, BF16, kind="ExternalOutput")

    with tile.TileContext(nc) as tc:
        with (
            tc.tile_pool(name="const", bufs=1) as cpool,
            tc.tile_pool(name="xt", bufs=1) as xpool,
            tc.tile_pool(name="kq", bufs=1) as kqpool,
            tc.tile_pool(name="vaug", bufs=1) as vpool,
            tc.tile_pool(name="wts", bufs=1) as wpool,
            tc.tile_pool(name="stream", bufs=2) as spool,
        ):
            # weights, loaded once on the Pool queue (parallel to sync queue)
            wq_s = wpool.tile([128, 8, DH], BF16, name="wq_s")
            wk_s = wpool.tile([128, 8, DH], BF16, name="wk_s")
            wv_s = wpool.tile([128, 8, DH], BF16, name="wv_s")
            wo_s = wpool.tile([128, 4, D], BF16, name="wo_s")
            # All loads go through the in-order SP queue: the scheduler's
            # coarsened cross-queue DMA waits serialize arbitrary pairs, so
            # explicit FIFO placement beats a second queue. Weights split in
            # 0.5MB chunks to keep any one hold on the DMA engines short.
            # wk first: K-projection chunks are emitted before Q's.
            def wload(dst, src):
                n = dst.shape[1]
                for c in range(0, n, n // 2):
                    nc.sync.dma_start(out=dst[:, c : c + n // 2, :],
                                      in_=src[:, c : c + n // 2, :])

            ctile = cpool.tile([128, 3, 128], BF16, name="ctile")
            cos_t = cpool.tile([128, S], BF16, name="cos_t")
            sin_t = cpool.tile([128, S], BF16, name="sin_t")
            pmat = ctile[:, 0, :]
            identb = ctile[:, 1, :]
            tri01 = ctile[:, 2, :]

            # x^T tiles: xt[dc] = [128 dims, S tokens], via XBAR DMA transpose.
            # The DMA order is latency-tuned: the scheduler adds coarsened
            # waits chaining each DMA to one a few slots earlier (even across
            # queues), so big transfers are interleaved between the x^T
            # chunks in the order compute first needs them.
            xt = [xpool.tile([128, S], BF16, name=f"xt{dc}") for dc in range(8)]

            def xtload(th, dc):
                t0 = th * THT
                nc.sync.dma_start_transpose(
                    out=xt[dc][:, t0 : t0 + THT],
                    in_=x[t0 : t0 + THT, dc * 128 : (dc + 1) * 128],
                )

            def half(dst, src, h):
                nc.sync.dma_start(out=dst[:, h * (S // 2) : (h + 1) * (S // 2)],
                                  in_=src[:, h * (S // 2) : (h + 1) * (S // 2)])

            nc.gpsimd.dma_start(out=ctile, in_=consts[:, :, :])
            half(cos_t, cosb, 0)
            half(sin_t, sinb, 0)
            wload(wk_s, wk)           # wk0, wk1
            for dc in range(8):
                xtload(0, dc)
            half(cos_t, cosb, 1)
            half(sin_t, sinb, 1)
            late_w = {0: (wq_s, wq), 1: (wv_s, wv), 2: (wo_s, wo)}
            for th in range(NTH):
                if th > 0:
                    for dc in range(8):
                        xtload(th, dc)
                if th in late_w:
                    dst, src = late_w[th]
                    wload(dst, src)

            # K^T / Q^T pair tiles: [128 dims (head 2p | head 2p+1), S tokens]
            kt_tiles = [kqpool.tile([128, S], BF16, name=f"ktp{p}") for p in range(NP)]
            qt_tiles = [kqpool.tile([128, S], BF16, name=f"qtp{p}") for p in range(NP)]
            ctx_tiles = [kqpool.tile([128, S], BF16, name=f"ctxp{p}") for p in range(NP)]
            # V tiles with ones column: [128 tokens, 8 heads, 64+1]
            v_tiles = [vpool.tile([128, 8, HD + 1], BF16, name=f"vt{t}") for t in range(NKT)]
            for t in range(NKT):
                # ones column via exp(0*x) = 1
                nc.scalar.activation(
                    v_tiles[t][:, :, HD], ctile[:, 0, 0:8],
                    mybir.ActivationFunctionType.Exp, scale=0.0,
                )

            with (
                tc.tile_pool(name="pst", bufs=2, space="PSUM") as pst,
                tc.tile_pool(name="pssc", bufs=2, space="PSUM") as pssc,
                tc.tile_pool(name="psav", bufs=2, space="PSUM") as psav,
            ):
                # ---------- work-item generators ----------
                def proj_chunks(th, defer=False):
                    t0 = th * THT
                    ts = slice(t0, t0 + THT)

                    def qk_mms(wsb, dst, p, pt):
                        # returns the rope-tail closure; caller emits it after
                        # the NEXT chunk's matmuls so the PE never waits on it
                        pool, tag = pt
                        acc = pool.tile([128, THT], F32, name="acc", tag=tag)
                        for dc in range(8):
                            nc.tensor.matmul(
                                acc, wsb[:, dc, p * 128 : (p + 1) * 128],
                                xt[dc][:, ts],
                                start=(dc == 0), stop=(dc == 7),
                            )
                        acc_sb = spool.tile([128, THT], BF16, name="acc_sb",
                                            tag="accsb", bufs=4)
                        nc.scalar.copy(acc_sb, acc)  # frees the PSUM slot fast
                        sacc = spool.tile([128, THT], BF16, name="sacc",
                                          tag="sacc", bufs=3)
                        nc.vector.tensor_mul(sacc, acc_sb, sin_t[:, ts])

                        def rope_tail():
                            # dst = C*acc + PM@(S*acc), S sign-folded host-side
                            rps = pool.tile([128, THT], F32, name="rps", tag=tag)
                            nc.tensor.matmul(rps, pmat, sacc)
                            t1 = spool.tile([128, THT], BF16, name="t1",
                                            tag="t1", bufs=3)
                            nc.vector.tensor_mul(t1, acc_sb, cos_t[:, ts])
                            nc.vector.tensor_add(dst[p][:, ts], t1, rps)
                        return rope_tail

                    chunks = []
                    late = []
                    tail_box = [None]
                    # in pass 0 the attention PSUM banks are still idle:
                    # alternate accumulators between the two pools so the
                    # ring depth doubles and chunks never wait on evacuation
                    cnt = [0]

                    def next_pt():
                        cnt[0] += 1
                        if th == 0 and cnt[0] <= 6 and cnt[0] % 2 == 0:
                            return (psav, "av")
                        return (pst, "tp")

                    def mk_qk(wsb, dst, p):
                        def qk_chunk(wsb=wsb, dst=dst, p=p, tail_box=tail_box):
                            prev = tail_box[0]
                            tail_box[0] = qk_mms(wsb, dst, p, next_pt())
                            if prev is not None:
                                prev()
                        return qk_chunk

                    if not defer:
                        for wsb, dst in ((wk_s, kt_tiles), (wq_s, qt_tiles)):
                            for p in range(NP):
                                chunks.append(mk_qk(wsb, dst, p))
                    else:
                        # early pairs stay in this phase; late pairs' K/Q
                        # become filler for the exp-bound next phase (they
                        # are only needed by that phase's late units)
                        for p in (0, 1):
                            chunks.append(mk_qk(wk_s, kt_tiles, p))
                            chunks.append(mk_qk(wq_s, qt_tiles, p))
                        for p in (2, 3):
                            late.append(mk_qk(wk_s, kt_tiles, p))
                            late.append(mk_qk(wq_s, qt_tiles, p))

                        def tail_flush(tail_box=tail_box):
                            prev = tail_box[0]
                            tail_box[0] = None
                            if prev is not None:
                                prev()
                        late.append(tail_flush)
                    for tl in range(THT // 128):
                        def v_chunk(tl=tl, t0=t0, th=th, tail_box=tail_box):
                            pool, tag = next_pt()
                            acc = pool.tile([128, DH], F32, name="vacc", tag=tag)
                            for dc in range(8):
                                nc.tensor.matmul(
                                    acc, xt[dc][:, t0 + tl * 128 : t0 + (tl + 1) * 128],
                                    wv_s[:, dc, :],
                                    start=(dc == 0), stop=(dc == 7),
                                )
                            prev = tail_box[0]
                            tail_box[0] = None
                            if prev is not None:
                                prev()
                            vt = v_tiles[th * (THT // 128) + tl]
                            nc.vector.tensor_copy(
                                vt[:, :, 0:HD],
                                acc.rearrange("a (h d) -> a h d", h=8),
                            )
                        chunks.append(v_chunk)
                    return (chunks, late) if defer else chunks

                def attn_iters(qb):
                    nk = 4 * qb + 4
                    q0 = qb * QB
                    iters = []

                    def av_group(p, qb, qci, eabs, ctxT_box):
                        # one contiguous accumulation group per (p, qci, head):
                        # the tile framework / PSUM HW mishandles interleaved
                        # groups within one tile, so never interleave them.
                        j = 4 * qb + qci
                        pseq = psav.tile([128, HD + 1], F32, name="pseq", tag="av")
                        psoq = psav.tile([128, HD + 1], F32, name="psoq", tag="av")
                        for kt2 in range(j + 1):
                            dj2 = max(0, kt2 - 4 * qb)
                            e0 = (qci - dj2) * 128
                            eab2 = eabs[kt2]
                            nc.tensor.matmul(
                                pseq, eab2[:, e0 : e0 + 128],
                                v_tiles[kt2][:, 2 * p, :],
                                start=(kt2 == 0), stop=(kt2 == j))
                            nc.tensor.matmul(
                                psoq, eab2[:, QB + e0 : QB + e0 + 128],
                                v_tiles[kt2][:, 2 * p + 1, :],
                                start=(kt2 == 0), stop=(kt2 == j))
                        rec = spool.tile([128, 2], F32, name="rec", tag="rec", bufs=4)
                        nc.vector.reciprocal(rec[:, 0:1], pseq[:, HD : HD + 1])
                        nc.vector.reciprocal(rec[:, 1:2], psoq[:, HD : HD + 1])
                        ctxT = spool.tile([128, 128], BF16, name="ctxT",
                                          tag="ctxT", bufs=8)
                        nc.vector.tensor_scalar_mul(ctxT[:, 0:HD], pseq[:, 0:HD],
                                                    rec[:, 0:1])
                        nc.vector.tensor_scalar_mul(ctxT[:, HD:128], psoq[:, 0:HD],
                                                    rec[:, 1:2])
                        tq = (4 * qb + qci) * 128
                        if qb == 3 and p == 3:
                            # tail-critical: PE transpose + DVE evac (~0.5us)
                            # instead of the ~2.3us XBAR DMA round trip
                            tr = psav.tile([128, 128], BF16, name="tr", tag="av")
                            nc.tensor.matmul(tr, ctxT, identb, is_transpose=True)
                            nc.vector.tensor_copy(
                                ctx_tiles[p][:, tq : tq + 128], tr)
                        else:
                            nc.sync.dma_start_transpose(
                                out=ctx_tiles[p][:, tq : tq + 128], in_=ctxT)

                    for p in range(NP):
                        iters.append([])
                        eabs = {}
                        ctxT_box = [None]
                        for j in range(nk):
                            def kt_iter(p=p, j=j, qb=qb, q0=q0, eabs=eabs,
                                        ctxT_box=ctxT_box):
                                kt = j
                                dj = kt - 4 * qb
                                qoff = 128 * dj if dj > 0 else 0
                                n = QB - qoff
                                ktp, qtp = kt_tiles[p], qt_tiles[p]
                                ksl = slice(kt * 128, (kt + 1) * 128)
                                qsl = slice(q0 + qoff, q0 + QB)
                                psab = pssc.tile([128, 2 * QB], F32, name="psab", tag="sc")
                                nc.tensor.matmul(psab[:, 0:n], ktp[0:64, ksl], qtp[0:64, qsl])
                                nc.tensor.matmul(psab[:, QB : QB + n],
                                                 ktp[64:128, ksl], qtp[64:128, qsl])
                                eab = spool.tile([128, 2 * QB], BF16, name="eab",
                                                 tag="eab", bufs=24)
                                eabs[kt] = eab
                                eview_o = bass.AP(
                                    tensor=eab.tensor, offset=eab.offset,
                                    ap=[eab.ap[0], [QB, 2], [1, n]])
                                eview_i = bass.AP(
                                    tensor=psab.tensor, offset=psab.offset,
                                    ap=[psab.ap[0], [QB, 2], [1, n]])
                                nc.scalar.activation(
                                    eview_o, eview_i,
                                    mybir.ActivationFunctionType.Exp, scale=SCALE)
                                if dj >= 0:
                                    # zero exp above the diagonal of the 128-chunk
                                    mview = bass.AP(
                                        tensor=eab.tensor, offset=eab.offset,
                                        ap=[eab.ap[0], [QB, 2], [1, 128]])
                                    tview = bass.AP(
                                        tensor=tri01.tensor, offset=tri01.offset,
                                        ap=[tri01.ap[0], [0, 2], [1, 128]])
                                    nc.vector.tensor_mul(mview, mview, tview)
                                if dbg and qb == 0 and p == 0 and kt == 0:
                                    nc.sync.dma_start(out=dbg["deab"][:, :], in_=eab[:, :])
                                # AV group one iteration behind: its last eab
                                # is already exp'd, so the PE never waits
                                if j - 1 >= 4 * qb:
                                    av_group(p, qb, j - 1 - 4 * qb, eabs, ctxT_box)
                            nmm = 2 * j if j - 1 >= 4 * qb else 0
                            npe = (2 * (QB - (128 * (j - 4 * qb) if j > 4 * qb else 0))
                                   + nmm * (HD + 1)) * 4.167e-4
                            nact = 2 * (QB - (128 * (j - 4 * qb) if j > 4 * qb else 0)) \
                                * 8.33e-4 + 0.37
                            iters[-1].append((kt_iter, npe, nact))

                        def p_flush(p=p, qb=qb, eabs=eabs, ctxT_box=ctxT_box):
                            av_group(p, qb, 3, eabs, ctxT_box)
                        iters[-1].append((p_flush, 2 * nk * (HD + 1) * 4.167e-4, 0.0))
                    return iters

                def oproj_chunk(t, nn):
                    def o_chunk(t=t, nn=nn):
                        oacc = pst.tile([128, 512], F32, name="oacc", tag="tp")
                        for pc in range(4):
                            nc.tensor.matmul(
                                oacc, ctx_tiles[pc][:, t * 128 : (t + 1) * 128],
                                wo_s[:, pc, nn * 512 : (nn + 1) * 512],
                                start=(pc == 0), stop=(pc == 3))
                        osb = spool.tile([128, 512], BF16, name="osb",
                                         tag="osb", bufs=6)
                        nc.vector.tensor_copy(osb, oacc)
                        nc.sync.dma_start(
                            out=outp[t * 128 : (t + 1) * 128,
                                     nn * 512 : (nn + 1) * 512], in_=osb)
                    return o_chunk

                # ---------- schedule ----------
                # qb=2/qb=3 attention units interleave so the exp-heavy tail
                # shares a region with enough PE filler; proj(th) chunks are
                # deadline-scheduled before attn(th) starts, out-proj tiles
                # gated on their query block's last pair finishing.
                def interleave(primary, filler):
                    # spread filler proportionally to each item's PE slack
                    wts = [1.0 if a - p_ > 0.5 else 0.25 for _, p_, a in primary]
                    total = sum(wts)
                    nf = len(filler)
                    fi, acc = 0, 0.0
                    for (fn, _, _), w in zip(primary, wts):
                        fn()
                        acc += w
                        while fi < min(int(acc / total * nf), nf):
                            filler[fi]()
                            fi += 1
                    while fi < nf:
                        filler[fi]()
                        fi += 1

                for c in proj_chunks(0):
                    c()
                aitems = {qb: attn_iters(qb) for qb in range(4)}

                def flat(qb):
                    return [it for pl in aitems[qb] for it in pl]

                interleave(flat(0), proj_chunks(1))
                p2_early, p2_late = proj_chunks(2, defer=True)
                interleave(flat(1), p2_early)
                p3_early, p3_late = proj_chunks(3, defer=True)
                interleave(flat(2), p2_late + p3_early)
                interleave(flat(3),
                           p3_late + [oproj_chunk(t, nn)
                                      for t in range(12) for nn in (0, 1)])
                for t in range(12, 16):
                    for nn in (0, 1):
                        oproj_chunk(t, nn)()

                if dbg:
                    nc.sync.dma_start(out=dbg["dxt0"][:, :], in_=xt[0][:, :])
                    nc.sync.dma_start(out=dbg["dkt0"][:, :], in_=kt_tiles[0][:, :])
                    nc.sync.dma_start(out=dbg["dqt0"][:, :], in_=qt_tiles[0][:, :])
                    nc.sync.dma_start(out=dbg["dv0"][:, :, :], in_=v_tiles[0][:, :, :])
                    nc.sync.dma_start(out=dbg["dctx0"][:, :], in_=ctx_tiles[0][:, :])

    nc.compile()
    return nc


def _host_tables(token_positions):
    pos = np.asarray(token_positions, dtype=np.float64)
    inv_freq = np.exp(np.arange(0, HD, 2, dtype=np.float64) * (-math.log(THETA) / HD))
    ang = pos[:, None] * inv_freq[None, :]  # [S, 32]
    cos = np.cos(ang).T  # [32, S]
    sin = np.sin(ang).T
    # pair-tile row layout: [head_even: 32 evens | 32 odds][head_odd: same]
    # sign folded so rope = C*acc + PM@(Sx*acc)
    C = np.empty((128, S), np.float64)
    Sx = np.empty((128, S), np.float64)
    for half in range(2):
        r0 = 64 * half
        C[r0 : r0 + 32] = cos
        C[r0 + 32 : r0 + 64] = cos
        Sx[r0 : r0 + 32] = sin
        Sx[r0 + 32 : r0 + 64] = -sin
    return C, Sx


def _host_consts():
    pm = np.zeros((128, 128), np.float64)
    for i in range(128):
        pm[i, i ^ 32] = 1.0
    ident = np.eye(128)
    tri = (np.arange(128)[None, :] >= np.arange(128)[:, None]).astype(np.float64)
    return np.stack([pm, ident, tri], axis=1)  # [128, 3, 128]


def kernel(in_features, token_positions, wq, wk, wv, wo):
    global _cached
    if _cached is None:
        _cached = _build()
    nc = _cached

    from ml_dtypes import bfloat16

    x = np.asarray(in_features, dtype=np.float32)
    # permute wq/wk columns within each head: [evens | odds]
    perm = np.concatenate(
        [64 * h + np.concatenate([np.arange(0, 64, 2), np.arange(1, 64, 2)]) for h in range(H)])
    wqp = np.asarray(wq, np.float32)[:, perm]
    wkp = np.asarray(wk, np.float32)[:, perm]
    wv = np.asarray(wv, np.float32)
    wo = np.asarray(wo, np.float32)
    C, Sx = _host_tables(token_positions)
    consts = _host_consts().astype(bfloat16)
    Cb = C.astype(bfloat16)
    Sb = Sx.astype(bfloat16)

    def wlayout(w):  # [1024, 512] -> [128, 8, 512] chunk-major
        return np.ascontiguousarray(
            w.reshape(8, 128, DH).transpose(1, 0, 2).astype(bfloat16))

    in_maps = []
    for c in range(8):
        b, g = c // 2, c % 2
        sl = slice(g * DH, (g + 1) * DH)
        wo_core = wo[sl, :]  # [512, 1024]
        in_maps.append({
            "x": np.ascontiguousarray(x[b].astype(bfloat16)),
            "wq": wlayout(wqp[:, sl]),
            "wk": wlayout(wkp[:, sl]),
            "wv": wlayout(wv[:, sl]),
            "wo": np.ascontiguousarray(
                wo_core.reshape(4, 128, D).transpose(1, 0, 2).astype(bfloat16)),
            "cosb": Cb,
            "sinb": Sb,
            "consts": consts,
        })
    results = _run(nc, in_maps)
    out = np.empty((B, S, D), np.float32)
    for b in range(B):
        out[b] = (results[2 * b]["outp"].astype(np.float32)
                  + results[2 * b + 1]["outp"].astype(np.float32))
    return out


_jit_cache = None


def _run(nc, in_maps):
    """Run the SPMD program on 8 cores, caching the jitted executable across
    calls (run_bass_kernel_spmd retraces every call). Falls back to the
    library path on any failure."""
    global _jit_cache
    try:
        import jax
        from jax.sharding import Mesh, PartitionSpec
        from jax.experimental.shard_map import shard_map
        from concourse import bass2jax
        import concourse.mybir as mybir

        if _jit_cache is None:
            bass2jax.install_neuronx_cc_hook()
            pid_name = nc.partition_id_tensor.name if nc.partition_id_tensor else None
            in_names, out_names, out_avals, zero_outs = [], [], [], []
            for alloc in nc.m.functions[0].allocations:
                if not isinstance(alloc, mybir.MemoryLocationSet):
                    continue
                nm = alloc.memorylocations[0].name
                if alloc.kind == "ExternalInput":
                    if nm != pid_name:
                        in_names.append(nm)
                elif alloc.kind == "ExternalOutput":
                    out_names.append(nm)
                    shape = tuple(alloc.tensor_shape)
                    dtype = mybir.dt.np(alloc.dtype)
                    out_avals.append(jax.core.ShapedArray(shape, dtype))
                    zero_outs.append(np.zeros(shape, dtype))
            n_params = len(in_names)
            all_names = in_names + out_names
            if pid_name is not None:
                all_names = all_names + [pid_name]

            def _body(*args):
                operands = list(args)
                if pid_name is not None:
                    operands.append(bass2jax.partition_id_tensor())
                outs = bass2jax._bass_exec_p.bind(
                    *operands, out_avals=tuple(out_avals), in_names=tuple(all_names),
                    out_names=tuple(out_names), lowering_input_output_aliases=(),
                    sim_require_finite=True, sim_require_nnan=True, nc=nc)
                return tuple(outs)

            devices = jax.devices()[:8]
            mesh = Mesh(np.asarray(devices), ("core",))
            nio = n_params + len(out_names)
            sharded = jax.jit(
                shard_map(_body, mesh=mesh, in_specs=(PartitionSpec("core"),) * nio,
                          out_specs=(PartitionSpec("core"),) * len(out_names),
                          check_rep=False),
                keep_unused=True)
            _jit_cache = (sharded, in_names, out_names, zero_outs)

        sharded, in_names, out_names, zero_outs = _jit_cache
        concat_in = [np.concatenate([np.asarray(m[nm]) for m in in_maps], axis=0)
                     for nm in in_names]
        concat_zero = [np.concatenate([z] * 8, axis=0) for z in zero_outs]
        outs = sharded(*concat_in, *concat_zero)
        results = []
        for c in range(8):
            d = {}
            for i, nm in enumerate(out_names):
                arr = np.asarray(outs[i])
                n0 = arr.shape[0] // 8
                d[nm] = arr[c * n0 : (c + 1) * n0]
            results.append(d)
        return results
    except Exception:
        res = run_bass_kernel_spmd(nc, in_maps, core_ids=list(range(8)))
        return res.results


# revision 49
# speedup vs baseline: 1.0422x; 1.0030x over previous
"""Causal MHA with RoPE on 8 Trainium2 NeuronCores.

Sharding: core c -> batch b=c//2, head-group g=c%2 (8 heads of 16).
Each core: Q/K/V projections for its 512 head-dims over the full sequence,
causal attention for its 8 heads, partial output projection (its 512 rows
of wo). Host sums the two partial outputs per batch. No collectives.

All operands bf16 (fp32 PSUM accumulation), prepared host-side:
 - x^T materialized by XBAR DMA-transpose straight into SBUF (no PE work).
 - Weights/tables loaded once, DMA order latency-tuned (the scheduler
   chains coarsened waits between nearby DMAs, so transfer sizes are kept
   small and ordered by first use).
 - RoPE: dst = C*acc + PM@(S*acc), PM a 32-row block-swap permutation
   matrix as a PE matmul (no SBUF swap DMAs); sign of S folded host-side;
   each chunk's rope tail is emitted after the next chunk's matmuls so the
   PE never waits on it.
 - Causal mask: exp first (ScalarE, scale=1/8 folded in), then one bf16
   DVE multiply of the diagonal 128-tile by a 0/1 lower-triangular mask.
 - AV computed transposed: exp-block stationary, [V | 1] moving ->
   ctx^T [q, dim] at 65 cols per (tile, head) instead of streaming exp
   twice; the ones column yields softmax denominators for free. Each
   (pair, query-chunk) accumulation group is contiguous and owns its PSUM
   tile: interleaved groups within one tile corrupt on hardware.
 - ctx^T scaled by 1/denom (per-partition scalar), transposed back to
   [dim, tok] by XBAR DMA for the output projection; bf16 output summed
   across head-group cores on the host.
Issue order interleaves projections of pass t+1 and the output projection
into the attention stream of pass t (weighted toward the exp-bound prefix
iterations) so ScalarE exp time hides behind PE work.
Timeline-sim: 242088 ns/core (baseline 413016, 1.71x); rel err 3.7e-3.
Late projection pairs are deferred into the following attention phase as
extra PE filler for its exp-bound prefix (they are only needed by that
phase's late pairs).
"""
import math
import os

import numpy as np

import concourse.bass as bass
import concourse.mybir as mybir
import concourse.tile as tile
from concourse import bacc
from concourse.bass_utils import run_bass_kernel_spmd

F32 = mybir.dt.float32
BF16 = mybir.dt.bfloat16

B, S, D, H = 4, 2048, 1024, 16
HD = D // H          # 64
THETA = 10000.0
DH = D // 2          # 512 per-core head dims (8 heads)
NP = 4               # head pairs per core
NTH = 4              # token passes
THT = S // NTH       # 512 tokens per pass
QB = THT             # query block
NKT = S // 128       # 16 key tiles of 128
SCALE = 1.0 / math.sqrt(HD)

_cached = None


def _build():
    nc = bacc.Bacc(None, target_bir_lowering=False)

    x = nc.dram_tensor("x", [S, D], BF16, kind="ExternalInput")
    wq = nc.dram_tensor("wq", [128, 8, DH], BF16, kind="ExternalInput")
    wk = nc.dram_tensor("wk", [128, 8, DH], BF16, kind="ExternalInput")
    wv = nc.dram_tensor("wv", [128, 8, DH], BF16, kind="ExternalInput")
    wo = nc.dram_tensor("wo", [128, 4, D], BF16, kind="ExternalInput")
    cosb = nc.dram_tensor("cosb", [128, S], BF16, kind="ExternalInput")
    sinb = nc.dram_tensor("sinb", [128, S], BF16, kind="ExternalInput")
    # [PM | ident | tri01] host-built constants
    consts = nc.dram_tensor("consts", [128, 3, 128], BF16, kind="ExternalInput")
    outp = nc.dram_tensor("outp", [S, D], BF16, kind="ExternalOutput")
    dbg = {}
    if os.environ.get("KDBG"):
        for nm, shp in (("dxt0", [128, S]), ("dkt0", [128, S]), ("dqt0", [128, S]),
                        ("dv0", [128, 8, HD + 1]), ("dctx0", [128, S]),
                        ("deab", [128, 2 * QB]), ("dpse", [128, 4, HD + 1]),
                        ("dctxT", [128, 4, 128])):
            dbg[nm] = nc.dram_tensor(nm, shp, BF16, kind="ExternalOutput")

    with tile.TileContext(nc) as tc:
        with (
            tc.tile_pool(name="const", bufs=1) as cpool,
            tc.tile_pool(name="xt", bufs=1) as xpool,
            tc.tile_pool(name="kq", bufs=1) as kqpool,
            tc.tile_pool(name="vaug", bufs=1) as vpool,
            tc.tile_pool(name="wts", bufs=1) as wpool,
            tc.tile_pool(name="stream", bufs=2) as spool,
        ):
            # weights, loaded once on the Pool queue (parallel to sync queue)
            wq_s = wpool.tile([128, 8, DH], BF16, name="wq_s")
            wk_s = wpool.tile([128, 8, DH], BF16, name="wk_s")
            wv_s = wpool.tile([128, 8, DH], BF16, name="wv_s")
            wo_s = wpool.tile([128, 4, D], BF16, name="wo_s")
            # All loads go through the in-order SP queue: the scheduler's
            # coarsened cross-queue DMA waits serialize arbitrary pairs, so
            # explicit FIFO placement beats a second queue. Weights split in
            # 0.5MB chunks to keep any one hold on the DMA engines short.
            # wk first: K-projection chunks are emitted before Q's.
            def wload(dst, src):
                n = dst.shape[1]
                for c in range(0, n, n // 2):
                    nc.sync.dma_start(out=dst[:, c : c + n // 2, :],
                                      in_=src[:, c : c + n // 2, :])

            ctile = cpool.tile([128, 3, 128], BF16, name="ctile")
            cos_t = cpool.tile([128, S], BF16, name="cos_t")
            sin_t = cpool.tile([128, S], BF16, name="sin_t")
            pmat = ctile[:, 0, :]
            identb = ctile[:, 1, :]
            tri01 = ctile[:, 2, :]

            # x^T tiles: xt[dc] = [128 dims, S tokens], via XBAR DMA transpose.
            # The DMA order is latency-tuned: the scheduler adds coarsened
            # waits chaining each DMA to one a few slots earlier (even across
            # queues), so big transfers are interleaved between the x^T
            # chunks in the order compute first needs them.
            xt = [xpool.tile([128, S], BF16, name=f"xt{dc}") for dc in range(8)]

            def xtload(th, dc):
                t0 = th * THT
                nc.sync.dma_start_transpose(
                    out=xt[dc][:, t0 : t0 + THT],
                    in_=x[t0 : t0 + THT, dc * 128 : (dc + 1) * 128],
                )

            def half(dst, src, h):
                nc.sync.dma_start(out=dst[:, h * (S // 2) : (h + 1) * (S // 2)],
                                  in_=src[:, h * (S // 2) : (h + 1) * (S // 2)])

            nc.gpsimd.dma_start(out=ctile, in_=consts[:, :, :])
            half(cos_t, cosb, 0)
            half(sin_t, sinb, 0)
            wload(wk_s, wk)           # wk0, wk1
            for dc in range(8):
                xtload(0, dc)
            half(cos_t, cosb, 1)
            half(sin_t, sinb, 1)
            late_w = {0: (wq_s, wq), 1: (wv_s, wv), 2: (wo_s, wo)}
            for th in range(NTH):
                if th > 0:
                    for dc in range(8):
                        xtload(th, dc)
                if th in late_w:
                    dst, src = late_w[th]
                    wload(dst, src)

            # K^T / Q^T pair tiles: [128 dims (head 2p | head 2p+1), S tokens]
            kt_tiles = [kqpool.tile([128, S], BF16, name=f"ktp{p}") for p in range(NP)]
            qt_tiles = [kqpool.tile([128, S], BF16, name=f"qtp{p}") for p in range(NP)]
            ctx_tiles = [kqpool.tile([128, S], BF16, name=f"ctxp{p}") for p in range(NP)]
            # V tiles with ones column: [128 tokens, 8 heads, 64+1]
            v_tiles = [vpool.tile([128, 8, HD + 1], BF16, name=f"vt{t}") for t in range(NKT)]
            for t in range(NKT):
                # ones column via exp(0*x) = 1
                nc.scalar.activation(
                    v_tiles[t][:, :, HD], ctile[:, 0, 0:8],
                    mybir.ActivationFunctionType.Exp, scale=0.0,
                )

            with (
                tc.tile_pool(name="pst", bufs=2, space="PSUM") as pst,
                tc.tile_pool(name="pssc", bufs=2, space="PSUM") as pssc,
                tc.tile_pool(name="psav", bufs=2, space="PSUM") as psav,
            ):
                # ---------- work-item generators ----------
                def proj_chunks(th, defer=False):
                    t0 = th * THT
                    ts = slice(t0, t0 + THT)

                    def qk_mms(wsb, dst, p, pt):
                        # returns the rope-tail closure; caller emits it after
                        # the NEXT chunk's matmuls so the PE never waits on it
                        pool, tag = pt
                        acc = pool.tile([128, THT], F32, name="acc", tag=tag)
                        for dc in range(8):
                            nc.tensor.matmul(
                                acc, wsb[:, dc, p * 128 : (p + 1) * 128],
                                xt[dc][:, ts],
                                start=(dc == 0), stop=(dc == 7),
                            )
                        acc_sb = spool.tile([128, THT], BF16, name="acc_sb",
                                            tag="accsb", bufs=4)
                        nc.scalar.copy(acc_sb, acc)  # frees the PSUM slot fast
                        sacc = spool.tile([128, THT], BF16, name="sacc",
                                          tag="sacc", bufs=3)
                        nc.vector.tensor_mul(sacc, acc_sb, sin_t[:, ts])

                        def rope_tail():
                            # dst = C*acc + PM@(S*acc), S sign-folded host-side
                            rps = pool.tile([128, THT], F32, name="rps", tag=tag)
                            nc.tensor.matmul(rps, pmat, sacc)
                            t1 = spool.tile([128, THT], BF16, name="t1",
                                            tag="t1", bufs=3)
                            nc.vector.tensor_mul(t1, acc_sb, cos_t[:, ts])
                            nc.vector.tensor_add(dst[p][:, ts], t1, rps)
                        return rope_tail

                    chunks = []
                    late = []
                    tail_box = [None]
                    # in pass 0 the attention PSUM banks are still idle:
                    # alternate accumulators between the two pools so the
                    # ring depth doubles and chunks never wait on evacuation
                    cnt = [0]

                    def next_pt():
                        cnt[0] += 1
                        if th == 0 and cnt[0] <= 6 and cnt[0] % 2 == 0:
                            return (psav, "av")
                        return (pst, "tp")

                    def mk_qk(wsb, dst, p):
                        def qk_chunk(wsb=wsb, dst=dst, p=p, tail_box=tail_box):
                            prev = tail_box[0]
                            tail_box[0] = qk_mms(wsb, dst, p, next_pt())
                            if prev is not None:
                                prev()
                        return qk_chunk

                    if not defer:
                        for wsb, dst in ((wk_s, kt_tiles), (wq_s, qt_tiles)):
                            for p in range(NP):
                                chunks.append(mk_qk(wsb, dst, p))
                    else:
                        # early pairs stay in this phase; late pairs' K/Q
                        # become filler for the exp-bound next phase (they
                        # are only needed by that phase's late units)
                        for p in (0, 1):
                            chunks.append(mk_qk(wk_s, kt_tiles, p))
                            chunks.append(mk_qk(wq_s, qt_tiles, p))
                        for p in (2, 3):
                            late.append(mk_qk(wk_s, kt_tiles, p))
                            late.append(mk_qk(wq_s, qt_tiles, p))

                        def tail_flush(tail_box=tail_box):
                            prev = tail_box[0]
                            tail_box[0] = None
                            if prev is not None:
                                prev()
                        late.append(tail_flush)
                    for tl in range(THT // 128):
                        def v_chunk(tl=tl, t0=t0, th=th, tail_box=tail_box):
                            pool, tag = next_pt()
                            acc = pool.tile([128, DH], F32, name="vacc", tag=tag)
                            for dc in range(8):
                                nc.tensor.matmul(
                                    acc, xt[dc][:, t0 + tl * 128 : t0 + (tl + 1) * 128],
                                    wv_s[:, dc, :],
                                    start=(dc == 0), stop=(dc == 7),
                                )
                            prev = tail_box[0]
                            tail_box[0] = None
                            if prev is not None:
                                prev()
                            vt = v_tiles[th * (THT // 128) + tl]
                            nc.vector.tensor_copy(
                                vt[:, :, 0:HD],
                                acc.rearrange("a (h d) -> a h d", h=8),
                            )
                        chunks.append(v_chunk)
                    return (chunks, late) if defer else chunks

                def attn_iters(qb):
                    nk = 4 * qb + 4
                    q0 = qb * QB
                    iters = []

                    def av_group(p, qb, qci, eabs, ctxT_box):
                        # one contiguous accumulation group per (p, qci, head):
                        # the tile framework / PSUM HW mishandles interleaved
                        # groups within one tile, so never interleave them.
                        j = 4 * qb + qci
                        pseq = psav.tile([128, HD + 1], F32, name="pseq", tag="av")
                        psoq = psav.tile([128, HD + 1], F32, name="psoq", tag="av")
                        for kt2 in range(j + 1):
                            dj2 = max(0, kt2 - 4 * qb)
                            e0 = (qci - dj2) * 128
                            eab2 = eabs[kt2]
                            nc.tensor.matmul(
                                pseq, eab2[:, e0 : e0 + 128],
                                v_tiles[kt2][:, 2 * p, :],
                                start=(kt2 == 0), stop=(kt2 == j))
                            nc.tensor.matmul(
                                psoq, eab2[:, QB + e0 : QB + e0 + 128],
                                v_tiles[kt2][:, 2 * p + 1, :],
                                start=(kt2 == 0), stop=(kt2 == j))
                        rec = spool.tile([128, 2], F32, name="rec", tag="rec", bufs=4)
                        nc.vector.reciprocal(rec[:, 0:1], pseq[:, HD : HD + 1])
                        nc.vector.reciprocal(rec[:, 1:2], psoq[:, HD : HD + 1])
                        ctxT = spool.tile([128, 128], BF16, name="ctxT",
                                          tag="ctxT", bufs=8)
                        nc.vector.tensor_scalar_mul(ctxT[:, 0:HD], pseq[:, 0:HD],
                                                    rec[:, 0:1])
                        nc.vector.tensor_scalar_mul(ctxT[:, HD:128], psoq[:, 0:HD],
                                                    rec[:, 1:2])
                        tq = (4 * qb + qci) * 128
                        if qb == 3 and p >= 2:
                            # tail-critical: PE transpose + DVE evac (~0.5us)
                            # instead of the ~2.3us XBAR DMA round trip
                            tr = psav.tile([128, 128], BF16, name="tr", tag="av")
                            nc.tensor.matmul(tr, ctxT, identb, is_transpose=True)
                            nc.vector.tensor_copy(
                                ctx_tiles[p][:, tq : tq + 128], tr)
                        else:
                            nc.sync.dma_start_transpose(
                                out=ctx_tiles[p][:, tq : tq + 128], in_=ctxT)

                    for p in range(NP):
                        iters.append([])
                        eabs = {}
                        ctxT_box = [None]
                        for j in range(nk):
                            def kt_iter(p=p, j=j, qb=qb, q0=q0, eabs=eabs,
                                        ctxT_box=ctxT_box):
                                kt = j
                                dj = kt - 4 * qb
                                qoff = 128 * dj if dj > 0 else 0
                                n = QB - qoff
                                ktp, qtp = kt_tiles[p], qt_tiles[p]
                                ksl = slice(kt * 128, (kt + 1) * 128)
                                qsl = slice(q0 + qoff, q0 + QB)
                                psab = pssc.tile([128, 2 * QB], F32, name="psab", tag="sc")
                                nc.tensor.matmul(psab[:, 0:n], ktp[0:64, ksl], qtp[0:64, qsl])
                                nc.tensor.matmul(psab[:, QB : QB + n],
                                                 ktp[64:128, ksl], qtp[64:128, qsl])
                                eab = spool.tile([128, 2 * QB], BF16, name="eab",
                                                 tag="eab", bufs=24)
                                eabs[kt] = eab
                                eview_o = bass.AP(
                                    tensor=eab.tensor, offset=eab.offset,
                                    ap=[eab.ap[0], [QB, 2], [1, n]])
                                eview_i = bass.AP(
                                    tensor=psab.tensor, offset=psab.offset,
                                    ap=[psab.ap[0], [QB, 2], [1, n]])
                                nc.scalar.activation(
                                    eview_o, eview_i,
                                    mybir.ActivationFunctionType.Exp, scale=SCALE)
                                if dj >= 0:
                                    # zero exp above the diagonal of the 128-chunk
                                    mview = bass.AP(
                                        tensor=eab.tensor, offset=eab.offset,
                                        ap=[eab.ap[0], [QB, 2], [1, 128]])
                                    tview = bass.AP(
                                        tensor=tri01.tensor, offset=tri01.offset,
                                        ap=[tri01.ap[0], [0, 2], [1, 128]])
                                    nc.vector.tensor_mul(mview, mview, tview)
                                if dbg and qb == 0 and p == 0 and kt == 0:
                                    nc.sync.dma_start(out=dbg["deab"][:, :], in_=eab[:, :])
                                # AV group one iteration behind: its last eab
                                # is already exp'd, so the PE never waits
                                if j - 1 >= 4 * qb:
                                    av_group(p, qb, j - 1 - 4 * qb, eabs, ctxT_box)
                            nmm = 2 * j if j - 1 >= 4 * qb else 0
                            npe = (2 * (QB - (128 * (j - 4 * qb) if j > 4 * qb else 0))
                                   + nmm * (HD + 1)) * 4.167e-4
                            nact = 2 * (QB - (128 * (j - 4 * qb) if j > 4 * qb else 0)) \
                                * 8.33e-4 + 0.37
                            iters[-1].append((kt_iter, npe, nact))

                        def p_flush(p=p, qb=qb, eabs=eabs, ctxT_box=ctxT_box):
                            av_group(p, qb, 3, eabs, ctxT_box)
                        iters[-1].append((p_flush, 2 * nk * (HD + 1) * 4.167e-4, 0.0))
                    return iters

                def oproj_chunk(t, nn):
                    def o_chunk(t=t, nn=nn):
                        oacc = pst.tile([128, 512], F32, name="oacc", tag="tp")
                        for pc in range(4):
                            nc.tensor.matmul(
                                oacc, ctx_tiles[pc][:, t * 128 : (t + 1) * 128],
                                wo_s[:, pc, nn * 512 : (nn + 1) * 512],
                                start=(pc == 0), stop=(pc == 3))
                        osb = spool.tile([128, 512], BF16, name="osb",
                                         tag="osb", bufs=6)
                        nc.vector.tensor_copy(osb, oacc)
                        nc.sync.dma_start(
                            out=outp[t * 128 : (t + 1) * 128,
                                     nn * 512 : (nn + 1) * 512], in_=osb)
                    return o_chunk

                # ---------- schedule ----------
                # qb=2/qb=3 attention units interleave so the exp-heavy tail
                # shares a region with enough PE filler; proj(th) chunks are
                # deadline-scheduled before attn(th) starts, out-proj tiles
                # gated on their query block's last pair finishing.
                def interleave(primary, filler):
                    # spread filler proportionally to each item's PE slack
                    wts = [1.0 if a - p_ > 0.5 else 0.25 for _, p_, a in primary]
                    total = sum(wts)
                    nf = len(filler)
                    fi, acc = 0, 0.0
                    for (fn, _, _), w in zip(primary, wts):
                        fn()
                        acc += w
                        while fi < min(int(acc / total * nf), nf):
                            filler[fi]()
                            fi += 1
                    while fi < nf:
                        filler[fi]()
                        fi += 1

                for c in proj_chunks(0):
                    c()
                aitems = {qb: attn_iters(qb) for qb in range(4)}

                def flat(qb):
                    return [it for pl in aitems[qb] for it in pl]

                interleave(flat(0), proj_chunks(1))
                p2_early, p2_late = proj_chunks(2, defer=True)
                interleave(flat(1), p2_early)
                p3_early, p3_late = proj_chunks(3, defer=True)
                interleave(flat(2), p2_late + p3_early)
                interleave(flat(3),
                           p3_late + [oproj_chunk(t, nn)
                                      for t in range(12) for nn in (0, 1)])
                for t in range(12, 16):
                    for nn in (0, 1):
                        oproj_chunk(t, nn)()

                if dbg:
                    nc.sync.dma_start(out=dbg["dxt0"][:, :], in_=xt[0][:, :])
                    nc.sync.dma_start(out=dbg["dkt0"][:, :], in_=kt_tiles[0][:, :])
                    nc.sync.dma_start(out=dbg["dqt0"][:, :], in_=qt_tiles[0][:, :])
                    nc.sync.dma_start(out=dbg["dv0"][:, :, :], in_=v_tiles[0][:, :, :])
                    nc.sync.dma_start(out=dbg["dctx0"][:, :], in_=ctx_tiles[0][:, :])

    nc.compile()
    return nc


def _host_tables(token_positions):
    pos = np.asarray(token_positions, dtype=np.float64)
    inv_freq = np.exp(np.arange(0, HD, 2, dtype=np.float64) * (-math.log(THETA) / HD))
    ang = pos[:, None] * inv_freq[None, :]  # [S, 32]
    cos = np.cos(ang).T  # [32, S]
    sin = np.sin(ang).T
    # pair-tile row layout: [head_even: 32 evens | 32 odds][head_odd: same]
    # sign folded so rope = C*acc + PM@(Sx*acc)
    C = np.empty((128, S), np.float64)
    Sx = np.empty((128, S), np.float64)
    for half in range(2):
        r0 = 64 * half
        C[r0 : r0 + 32] = cos
        C[r0 + 32 : r0 + 64] = cos
        Sx[r0 : r0 + 32] = sin
        Sx[r0 + 32 : r0 + 64] = -sin
    return C, Sx


def _host_consts():
    pm = np.zeros((128, 128), np.float64)
    for i in range(128):
        pm[i, i ^ 32] = 1.0
    ident = np.eye(128)
    tri = (np.arange(128)[None, :] >= np.arange(128)[:, None]).astype(np.float64)
    return np.stack([pm, ident, tri], axis=1)  # [128, 3, 128]


def kernel(in_features, token_positions, wq, wk, wv, wo):
    global _cached
    if _cached is None:
        _cached = _build()
    nc = _cached

    from ml_dtypes import bfloat16

    x = np.asarray(in_features, dtype=np.float32)
    # permute wq/wk columns within each head: [evens | odds]
    perm = np.concatenate(
        [64 * h + np.concatenate([np.arange(0, 64, 2), np.arange(1, 64, 2)]) for h in range(H)])
    wqp = np.asarray(wq, np.float32)[:, perm]
    wkp = np.asarray(wk, np.float32)[:, perm]
    wv = np.asarray(wv, np.float32)
    wo = np.asarray(wo, np.float32)
    C, Sx = _host_tables(token_positions)
    consts = _host_consts().astype(bfloat16)
    Cb = C.astype(bfloat16)
    Sb = Sx.astype(bfloat16)

    def wlayout(w):  # [1024, 512] -> [128, 8, 512] chunk-major
        return np.ascontiguousarray(
            w.reshape(8, 128, DH).transpose(1, 0, 2).astype(bfloat16))

    in_maps = []
    for c in range(8):
        b, g = c // 2, c % 2
        sl = slice(g * DH, (g + 1) * DH)
        wo_core = wo[sl, :]  # [512, 1024]
        in_maps.append({
            "x": np.ascontiguousarray(x[b].astype(bfloat16)),
            "wq": wlayout(wqp[:, sl]),
            "wk": wlayout(wkp[:, sl]),
            "wv": wlayout(wv[:, sl]),
            "wo": np.ascontiguousarray(
                wo_core.reshape(4, 128, D).transpose(1, 0, 2).astype(bfloat16)),
            "cosb": Cb,
            "sinb": Sb,
            "consts": consts,
        })
    results = _run(nc, in_maps)
    out = np.empty((B, S, D), np.float32)
    for b in range(B):
        out[b] = (results[2 * b]["outp"].astype(np.float32)
                  + results[2 * b + 1]["outp"].astype(np.float32))
    return out


_jit_cache = None


def _run(nc, in_maps):
    """Run the SPMD program on 8 cores, caching the jitted executable across
    calls (run_bass_kernel_spmd retraces every call). Falls back to the
    library path on any failure."""
    global _jit_cache
    try:
        import jax
        from jax.sharding import Mesh, PartitionSpec
        from jax.experimental.shard_map import shard_map
        from concourse import bass2jax
        import concourse.mybir as mybir

        if _jit_cache is None:
            bass2jax.install_neuronx_cc_hook()
            pid_name = nc.partition_id_tensor.name if nc.partition_id_tensor else None
            in_names, out_names, out_avals, zero_outs = [], [], [], []
            for alloc in nc.m.functions[0].allocations:
                if not isinstance(alloc, mybir.MemoryLocationSet):
                    continue
                nm = alloc.memorylocations[0].name
                if alloc.kind == "ExternalInput":
                    if nm != pid_name:
                        in_names.append(nm)
                elif alloc.kind == "ExternalOutput":
                    out_names.append(nm)
                    shape = tuple(alloc.tensor_shape)
                    dtype = mybir.dt.np(alloc.dtype)
                    out_avals.append(jax.core.ShapedArray(shape, dtype))
                    zero_outs.append(np.zeros(shape, dtype))
            n_params = len(in_names)
            all_names = in_names + out_names
            if pid_name is not None:
                all_names = all_names + [pid_name]

            def _body(*args):
                operands = list(args)
                if pid_name is not None:
                    operands.append(bass2jax.partition_id_tensor())
                outs = bass2jax._bass_exec_p.bind(
                    *operands, out_avals=tuple(out_avals), in_names=tuple(all_names),
                    out_names=tuple(out_names), lowering_input_output_aliases=(),
                    sim_require_finite=True, sim_require_nnan=True, nc=nc)
                return tuple(outs)

            devices = jax.devices()[:8]
            mesh = Mesh(np.asarray(devices), ("core",))
            nio = n_params + len(out_names)
            sharded = jax.jit(
                shard_map(_body, mesh=mesh, in_specs=(PartitionSpec("core"),) * nio,
                          out_specs=(PartitionSpec("core"),) * len(out_names),
                          check_rep=False),
                keep_unused=True)
            _jit_cache = (sharded, in_names, out_names, zero_outs)

        sharded, in_names, out_names, zero_outs = _jit_cache
        concat_in = [np.concatenate([np.asarray(m[nm]) for m in in_maps], axis=0)
                     for nm in in_names]
        concat_zero = [np.concatenate([z] * 8, axis=0) for z in zero_outs]
        outs = sharded(*concat_in, *concat_zero)
        results = []
        for c in range(8):
            d = {}
            for i, nm in enumerate(out_names):
                arr = np.asarray(outs[i])
                n0 = arr.shape[0] // 8
                d[nm] = arr[c * n0 : (c + 1) * n0]
            results.append(d)
        return results
    except Exception:
        res = run_bass_kernel_spmd(nc, in_maps, core_ids=list(range(8)))
        return res.results
